# revision 1
# baseline (speedup 1.0000x reference)
"""Trainium2 Bass kernel for nn_EvenOddFunctionHAM.

Computes, for W = W_tensor * W_mask (block-staircase 4096x4096):
    s_odd = rho(s) @ W + b_odd;  s_odd[:, :2048] += Ux
    out   = rho(s_odd) @ W.T + b_even
with rho(x) = sigmoid(4x - 2).

Strategy: data-parallel over the batch (4096 rows -> 8 cores x 512).
Everything runs in a transposed layout (feature dim on SBUF partitions,
batch on the free axis) so no on-device transposes are needed:
    S1 = W.T @ rho(s).T   (contraction over the even dim)
    O  = W  @ rho(S1+..)  (contraction over the odd dim)
Weights are masked, cast to fp16, transposed, and pre-tiled into
contiguous per-m-tile K-strips on the host; matmuls run in fp16 with
fp32 PSUM accumulation. The staircase zero block is skipped when the
masked W actually has it (checked on host), saving 25% of the FLOPs.
"""

import numpy as np

_KERNEL_CACHE = {}

_DEFAULT_OPTS = {
    "ring_split": True,
    "mm1_odd0_first": True,
    "psum_bufs": 8,
    # fuse_ldw=True re-fuses Ldweights into self-loading matmuls and enables
    # walrus --enable-ldw-opt. Measured perf-neutral on this kernel (the PE
    # stream is not LDW-bound), so keep the default, battle-tested compile
    # path.
    "fuse_ldw": False,
    "wpool_bufs": 3,
    "stage_bufs": 4,
    "strip_gpsimd": False,
    # Stream s.T / Ux.T as fp16 (saves 6 MiB/core HBM traffic). Measured
    # perf-neutral at R=25 (554 vs 561 us) with slightly worse rel err
    # (3.27e-4 vs 3.08e-4), so keep fp32 inputs.
    "io_f16": False,
}

# ---- model dims (hardcoded per contract; asserted against inputs) ----
B = 4096        # batch
E = 4096        # even dim (rows of W)
O_DIM = 4096    # odd dim (cols of W)
D1 = 2048       # width of Ux / first odd block
NC = 8          # cores
BC = B // NC    # batch per core = 512
P = 128         # partitions
NKE = E // P    # 32 k-tiles over even
NKO = O_DIM // P
NM1 = O_DIM // P  # mm1 output tiles (odd)
NM2 = E // P      # mm2 output tiles (even)
HALF = D1 // P    # 16


def _split_excess_waits(nc, maxw: int = 1) -> int:
    """This walrus build encodes at most one sem wait per instruction, but
    Tile's scheduler can attach several. Move the overflow onto inserted
    same-engine NoOps directly preceding the instruction (engines are
    in-order, so consecutive waits are equivalent to one multi-wait)."""
    from concourse import mybir

    n = 0
    for f in nc.m.functions:
        for bb in f.blocks:
            insts = bb.instructions
            new = []
            for inst in insts:
                si = getattr(inst, "sync_info", None)
                if si is not None and len(si.on_wait) > maxw:
                    waits = list(si.on_wait)
                    over, keep = waits[:-maxw], waits[-maxw:]
                    for j in range(0, len(over), maxw):
                        n += 1
                        new.append(mybir.InstNoOp(
                            name=f"{inst.name}-ws{j}",
                            engine=inst.engine,
                            bass_nofuse=True,
                            sync_info=mybir.SyncInfo(
                                on_wait=over[j : j + maxw], on_update=[]
                            ),
                        ))
                    inst.sync_info = mybir.SyncInfo(
                        on_wait=keep, on_update=list(si.on_update)
                    )
                new.append(inst)
            if len(new) != len(insts):
                insts[:] = new
                assert len(bb.instructions) == len(new)
    return n



_LDW_PATCHED = False


def _patch_ldw_opt():
    """Compile with walrus --enable-ldw-opt=true (the concourse default
    pins it false). Requires self-loading matmuls (no explicit
    InstLdweights), which _fuse_ldweights produces."""
    global _LDW_PATCHED
    if _LDW_PATCHED:
        return
    from concourse import bass_utils
    _orig = bass_utils.run_command

    def _patched(argv, **kwargs):
        argv = ["--enable-ldw-opt=true" if a == "--enable-ldw-opt=false" else a
                for a in argv]
        return _orig(argv, **kwargs)

    bass_utils.run_command = _patched
    _LDW_PATCHED = True


def _fuse_ldweights(nc) -> int:
    """Tile legalization splits each matmul into InstLdweights + InstMatmult.
    Walrus's LDW optimization (fast weight load + pipelining) only applies to
    self-loading matmuls, so re-fuse: drop the Ldweights, move its sem waits
    onto the matmul, set ldweights=True."""
    from concourse import mybir

    n = 0
    for f in nc.m.functions:
        for bb in f.blocks:
            insts = bb.instructions
            new, pending = [], None
            for inst in insts:
                tn = type(inst).__name__
                if tn == "InstLdweights":
                    assert pending is None
                    pending = inst
                    continue
                if tn == "InstMatmult" and pending is not None:
                    si_l, si_m = pending.sync_info, inst.sync_info
                    waits = list(si_l.on_wait if si_l else []) + \
                        list(si_m.on_wait if si_m else [])
                    ups = list(si_l.on_update if si_l else []) + \
                        list(si_m.on_update if si_m else [])
                    inst.sync_info = mybir.SyncInfo(on_wait=waits, on_update=ups)
                    inst.ldweights = True
                    pending = None
                    n += 1
                new.append(inst)
            assert pending is None
            if len(new) != len(insts):
                insts[:] = new
    return n


def _build(sparse: bool, reps: int = 1, opts: dict | None = None, split_waits: bool = True):
    """Build the per-core Bass program (same program on all 8 cores).

    reps > 1 replicates the whole computation back-to-back inside one NEFF
    (output overwritten each rep) — used only for differential timing."""
    opts = dict(_DEFAULT_OPTS, **(opts or {}))
    import concourse.bass as bass
    import concourse.tile as tile
    from concourse import mybir

    f32 = mybir.dt.float32
    f16 = mybir.dt.float16

    nk1a = HALF if sparse else NKE   # mm1 K-tiles for odd0 m-tiles
    nk2b = HALF if sparse else NKO   # mm2 K-tiles for even1 m-tiles

    nc = bass.Bass("TRN2", target_bir_lowering=False, debug=False)

    io_dt = f16 if opts["io_f16"] else f32
    sT = nc.dram_tensor("sT", [NKE, P, BC], io_dt, kind="ExternalInput")
    uT = nc.dram_tensor("uT", [HALF, P, BC], io_dt, kind="ExternalInput")
    w1a = nc.dram_tensor("w1a", [HALF, P, nk1a, P], f16, kind="ExternalInput")
    w1b = nc.dram_tensor("w1b", [HALF, P, NKE, P], f16, kind="ExternalInput")
    w2a = nc.dram_tensor("w2a", [HALF, P, NKO, P], f16, kind="ExternalInput")
    w2b = nc.dram_tensor("w2b", [HALF, P, nk2b, P], f16, kind="ExternalInput")
    bias1 = nc.dram_tensor("bias1", [P, NM1], f32, kind="ExternalInput")
    bias2 = nc.dram_tensor("bias2", [P, NM2], f32, kind="ExternalInput")
    out = nc.dram_tensor("o", [NM2, P, BC], f32, kind="ExternalOutput")

    with tile.TileContext(nc) as tc:
        with (
            tc.tile_pool(name="consts", bufs=1) as consts,
            tc.tile_pool(name="stage", bufs=opts["stage_bufs"]) as stage,
            tc.tile_pool(name="at", bufs=NKE) as at_pool,
            tc.tile_pool(name="ut", bufs=HALF) as ut_pool,
            tc.tile_pool(name="a2", bufs=NKO) as a2_pool,
            tc.tile_pool(name="wpool", bufs=opts["wpool_bufs"]) as wpool,
            tc.tile_pool(name="psum", bufs=opts["psum_bufs"], space="PSUM") as psum_pool,
            tc.tile_pool(name="opool", bufs=4) as opool,
        ):
            b1 = consts.tile([P, NM1], f32, tag="b1")
            nc.sync.dma_start(out=b1, in_=bias1[:, :])
            b2 = consts.tile([P, NM2], f32, tag="b2")
            nc.sync.dma_start(out=b2, in_=bias2[:, :])
            bneg2 = consts.tile([P, 1], f32, tag="bneg2")
            nc.vector.memset(bneg2, -2.0)

            pools = dict(
                stage=stage, at_pool=at_pool, ut_pool=ut_pool,
                a2_pool=a2_pool, wpool=wpool, psum_pool=psum_pool,
                opool=opool,
            )
            drams = dict(
                sT=sT, uT=uT, w1a=w1a, w1b=w1b, w2a=w2a, w2b=w2b, out=out
            )
            biases = dict(b1=b1, b2=b2, bneg2=bneg2)
            for _rep in range(reps):
                _kernel_body(nc, mybir, sparse, pools, drams, biases, opts)
    if opts["fuse_ldw"]:
        _patch_ldw_opt()
        _fuse_ldweights(nc)
    if split_waits:
        _split_excess_waits(nc, 1)
    return nc


def _kernel_body(nc, mybir, sparse, pools, drams, biases, opts):
    f32 = mybir.dt.float32
    f16 = mybir.dt.float16
    nk1a = HALF if sparse else NKE
    nk2b = HALF if sparse else NKO
    stage, at_pool, ut_pool, a2_pool, wpool, psum_pool, opool = (
        pools["stage"], pools["at_pool"], pools["ut_pool"], pools["a2_pool"],
        pools["wpool"], pools["psum_pool"], pools["opool"],
    )
    sT, uT, w1a, w1b, w2a, w2b, out = (
        drams["sT"], drams["uT"], drams["w1a"], drams["w1b"], drams["w2a"],
        drams["w2b"], drams["out"],
    )
    b1, b2, bneg2 = biases["b1"], biases["b2"], biases["bneg2"]
    io_dt = f16 if opts["io_f16"] else f32
    act_dma = nc.scalar if opts["ring_split"] else nc.sync
    strip_dma = nc.gpsimd if opts["strip_gpsimd"] else nc.sync

    # DMA ring split: weight strips go on the SP HWDGE ring (nc.sync),
    # activations/outputs on the ACT HWDGE ring (nc.scalar) — so the bulk
    # sT/uT loads never head-of-line-block the strip prefetch FIFO.

    # rho(s).T tiles, fp16, resident: AT[k] = sigmoid(4*sT[k] - 2)
    AT = []
    for k in range(NKE):
        st = stage.tile([P, BC], io_dt, tag="stage")
        act_dma.dma_start(out=st, in_=sT[k])
        a = at_pool.tile([P, BC], f16, tag="at")
        nc.scalar.activation(
            a, st, mybir.ActivationFunctionType.Sigmoid,
            bias=bneg2[:, 0:1], scale=4.0,
        )
        AT.append(a)

    # Ux.T tiles, fp32, resident
    UT = []
    for k in range(HALF):
        u = ut_pool.tile([P, BC], io_dt, tag="ut")
        act_dma.dma_start(out=u, in_=uT[k])
        UT.append(u)

    # ---- mm1: S1[odd,:] = W.T @ AT ; A2 = rho(S1 + b_odd [+ U]) ----
    # odd0 first: those m-tiles contract only over even0 (AT[0:16]), so the
    # PE can start after ~1/3 of the sT load instead of all of it.
    A2 = [None] * NM1
    mm1_order = (list(range(NM1)) if opts["mm1_odd0_first"]
                 else list(range(HALF, NM1)) + list(range(HALF)))
    for m in mm1_order:
        if m >= HALF:
            wt = wpool.tile([P, NKE, P], f16, tag="w")
            strip_dma.dma_start(out=wt, in_=w1b[m - HALF])
            ks = range(NKE)
        else:
            wt = wpool.tile([P, nk1a, P], f16, tag="w")
            strip_dma.dma_start(out=wt, in_=w1a[m])
            ks = range(nk1a)
        ps = psum_pool.tile([P, BC], f32, tag="ps")
        nkl = len(ks)
        for i, k in enumerate(ks):
            nc.tensor.matmul(
                ps, lhsT=wt[:, i, :], rhs=AT[k],
                start=(i == 0), stop=(i == nkl - 1),
            )
        if m < HALF:
            nc.vector.tensor_add(ps, ps, UT[m])
        a2 = a2_pool.tile([P, BC], f16, tag="a2")
        nc.scalar.activation(
            a2, ps, mybir.ActivationFunctionType.Sigmoid,
            bias=b1[:, m : m + 1], scale=4.0,
        )
        A2[m] = a2

    # ---- mm2: O[even,:] = W @ A2 + b_even ----
    # even1 first (small strips, deps = A2[16:] = the tail of mm1).
    for m in list(range(HALF, NM2)) + list(range(HALF)):
        if m >= HALF:
            wt = wpool.tile([P, nk2b, P], f16, tag="w")
            strip_dma.dma_start(out=wt, in_=w2b[m - HALF])
            ks = range(NKO - nk2b, NKO)
        else:
            wt = wpool.tile([P, NKO, P], f16, tag="w")
            strip_dma.dma_start(out=wt, in_=w2a[m])
            ks = range(NKO)
        ps = psum_pool.tile([P, BC], f32, tag="ps")
        nkl = len(ks)
        for i, k in enumerate(ks):
            nc.tensor.matmul(
                ps, lhsT=wt[:, i, :], rhs=A2[k],
                start=(i == 0), stop=(i == nkl - 1),
            )
        ot = opool.tile([P, BC], f32, tag="ot")
        nc.scalar.activation(
            ot, ps, mybir.ActivationFunctionType.Identity,
            bias=b2[:, m : m + 1], scale=1.0,
        )
        act_dma.dma_start(out=out[m], in_=ot)


def _strips(Wsub: np.ndarray, nm: int) -> np.ndarray:
    """[K, nm*128] -> [nm, 128, K//128, 128] contiguous per-m-tile K-strips.

    strip[j, p, kt, c] = Wsub[kt*128 + p, j*128 + c], so strip[j][:, kt, :]
    is the [K=128, M=128] lhsT tile for output tile j, contraction tile kt.
    """
    K = Wsub.shape[0]
    return np.ascontiguousarray(
        Wsub.reshape(K // P, P, nm, P).transpose(2, 1, 0, 3)
    )


def prepare_in_maps(inputs: dict, W: np.ndarray, sparse: bool, io_f16: bool = True) -> list:
    """Host-side prep: mask+cast+tile weights, transpose activations, shard."""
    f32 = np.float32
    s = np.asarray(inputs["s"], f32)
    Ux = np.asarray(inputs["Ux"], f32)
    assert s.shape == (B, E) and Ux.shape == (B, D1), (s.shape, Ux.shape)

    W16 = W.astype(np.float16)
    WT16 = np.ascontiguousarray(W16.T)

    if sparse:
        w1a = _strips(W16[:D1, :D1], HALF)
        w2b = _strips(WT16[D1:, D1:], HALF)
    else:
        w1a = _strips(W16[:, :D1], HALF)
        w2b = _strips(WT16[:, D1:], HALF)
    w1b = _strips(W16[:, D1:], HALF)
    w2a = _strips(WT16[:, :D1], HALF)

    bias1 = np.ascontiguousarray(
        (4.0 * np.asarray(inputs["b_odd"], f32).reshape(-1) - 2.0).reshape(NM1, P).T
    )
    bias2 = np.ascontiguousarray(
        np.asarray(inputs["b_even"], f32).reshape(-1).reshape(NM2, P).T
    )

    io_dt = np.float16 if io_f16 else f32
    sT_full = np.ascontiguousarray(s.T.astype(io_dt))   # [E, B]
    uT_full = np.ascontiguousarray(Ux.T.astype(io_dt))  # [D1, B]

    in_maps = []
    for c in range(NC):
        sl = slice(c * BC, (c + 1) * BC)
        in_maps.append({
            "sT": np.ascontiguousarray(sT_full[:, sl]).reshape(NKE, P, BC),
            "uT": np.ascontiguousarray(uT_full[:, sl]).reshape(HALF, P, BC),
            "w1a": w1a, "w1b": w1b, "w2a": w2a, "w2b": w2b,
            "bias1": bias1, "bias2": bias2,
        })
    return in_maps


def kernel(Ux, s, W_tensor, b_even, b_odd, W_mask):
    from concourse.bass_utils import run_bass_kernel_spmd

    f32 = np.float32
    W = np.asarray(W_tensor, f32) * np.asarray(W_mask, f32)
    sparse = not W[D1:, :D1].any()

    in_maps = prepare_in_maps(
        {"s": s, "Ux": Ux, "b_odd": b_odd, "b_even": b_even}, W, sparse,
        io_f16=_DEFAULT_OPTS["io_f16"],
    )

    nc = _KERNEL_CACHE.get(sparse)
    if nc is None:
        nc = _build(sparse)
        _KERNEL_CACHE[sparse] = nc

    res = run_bass_kernel_spmd(nc, in_maps, core_ids=list(range(NC)))
    out_T = np.concatenate(
        [res.results[c]["o"].reshape(E, BC) for c in range(NC)], axis=1
    )  # [E, B]
    return np.ascontiguousarray(out_T.T)



# revision 2
# speedup vs baseline: 1.0128x; 1.0128x over previous
"""Trainium2 Bass kernel for nn_EvenOddFunctionHAM.

Computes, for W = W_tensor * W_mask (block-staircase 4096x4096):
    s_odd = rho(s) @ W + b_odd;  s_odd[:, :2048] += Ux
    out   = rho(s_odd) @ W.T + b_even
with rho(x) = sigmoid(4x - 2).

Strategy: data-parallel over the batch (4096 rows -> 8 cores x 512).
Everything runs in a transposed layout (feature dim on SBUF partitions,
batch on the free axis) so no on-device transposes are needed:
    S1 = W.T @ rho(s).T   (contraction over the even dim)
    O  = W  @ rho(S1+..)  (contraction over the odd dim)
Weights are masked, cast to fp16, transposed, and pre-tiled into
contiguous per-m-tile K-strips on the host; matmuls run in fp16 with
fp32 PSUM accumulation. The staircase zero block is skipped when the
masked W actually has it (checked on host), saving 25% of the FLOPs.
"""

import numpy as np

_KERNEL_CACHE = {}

_DEFAULT_OPTS = {
    "ring_split": True,
    "mm1_odd0_first": True,
    "psum_bufs": 8,
    # fuse_ldw=True re-fuses Ldweights into self-loading matmuls and enables
    # walrus --enable-ldw-opt. Measured perf-neutral on this kernel (the PE
    # stream is not LDW-bound), so keep the default, battle-tested compile
    # path.
    "fuse_ldw": False,
    "wpool_bufs": 3,
    "stage_bufs": 4,
    "strip_gpsimd": False,
    # Stream s.T / Ux.T as fp16 (saves 6 MiB/core HBM traffic). Measured
    # perf-neutral at R=25 (554 vs 561 us) with slightly worse rel err
    # (3.27e-4 vs 3.08e-4), so keep fp32 inputs.
    "io_f16": False,
    # v2 schedule: fp16 in/out streams, resident stage pool, uT before the
    # even1 half of sT, and a2 activations interleaved ahead of the even1 AT
    # conversions so PSUM banks free promptly (the v1 act-queue FIFO stalled
    # the PE ~16us per rep waiting on stage DMAs).
    "v2": False,
    "out_f16": True,
}

# ---- model dims (hardcoded per contract; asserted against inputs) ----
B = 4096        # batch
E = 4096        # even dim (rows of W)
O_DIM = 4096    # odd dim (cols of W)
D1 = 2048       # width of Ux / first odd block
NC = 8          # cores
BC = B // NC    # batch per core = 512
P = 128         # partitions
NKE = E // P    # 32 k-tiles over even
NKO = O_DIM // P
NM1 = O_DIM // P  # mm1 output tiles (odd)
NM2 = E // P      # mm2 output tiles (even)
HALF = D1 // P    # 16


def _split_excess_waits(nc, maxw: int = 1) -> int:
    """This walrus build encodes at most one sem wait per instruction, but
    Tile's scheduler can attach several. Move the overflow onto inserted
    same-engine NoOps directly preceding the instruction (engines are
    in-order, so consecutive waits are equivalent to one multi-wait)."""
    from concourse import mybir

    n = 0
    for f in nc.m.functions:
        for bb in f.blocks:
            insts = bb.instructions
            new = []
            for inst in insts:
                si = getattr(inst, "sync_info", None)
                if si is not None and len(si.on_wait) > maxw:
                    waits = list(si.on_wait)
                    over, keep = waits[:-maxw], waits[-maxw:]
                    for j in range(0, len(over), maxw):
                        n += 1
                        new.append(mybir.InstNoOp(
                            name=f"{inst.name}-ws{j}",
                            engine=inst.engine,
                            bass_nofuse=True,
                            sync_info=mybir.SyncInfo(
                                on_wait=over[j : j + maxw], on_update=[]
                            ),
                        ))
                    inst.sync_info = mybir.SyncInfo(
                        on_wait=keep, on_update=list(si.on_update)
                    )
                new.append(inst)
            if len(new) != len(insts):
                insts[:] = new
                assert len(bb.instructions) == len(new)
    return n



_LDW_PATCHED = False


def _patch_ldw_opt():
    """Compile with walrus --enable-ldw-opt=true (the concourse default
    pins it false). Requires self-loading matmuls (no explicit
    InstLdweights), which _fuse_ldweights produces."""
    global _LDW_PATCHED
    if _LDW_PATCHED:
        return
    from concourse import bass_utils
    _orig = bass_utils.run_command

    def _patched(argv, **kwargs):
        argv = ["--enable-ldw-opt=true" if a == "--enable-ldw-opt=false" else a
                for a in argv]
        return _orig(argv, **kwargs)

    bass_utils.run_command = _patched
    _LDW_PATCHED = True


def _fuse_ldweights(nc) -> int:
    """Tile legalization splits each matmul into InstLdweights + InstMatmult.
    Walrus's LDW optimization (fast weight load + pipelining) only applies to
    self-loading matmuls, so re-fuse: drop the Ldweights, move its sem waits
    onto the matmul, set ldweights=True."""
    from concourse import mybir

    n = 0
    for f in nc.m.functions:
        for bb in f.blocks:
            insts = bb.instructions
            new, pending = [], None
            for inst in insts:
                tn = type(inst).__name__
                if tn == "InstLdweights":
                    assert pending is None
                    pending = inst
                    continue
                if tn == "InstMatmult" and pending is not None:
                    si_l, si_m = pending.sync_info, inst.sync_info
                    waits = list(si_l.on_wait if si_l else []) + \
                        list(si_m.on_wait if si_m else [])
                    ups = list(si_l.on_update if si_l else []) + \
                        list(si_m.on_update if si_m else [])
                    inst.sync_info = mybir.SyncInfo(on_wait=waits, on_update=ups)
                    inst.ldweights = True
                    pending = None
                    n += 1
                new.append(inst)
            assert pending is None
            if len(new) != len(insts):
                insts[:] = new
    return n


def _build(sparse: bool, reps: int = 1, opts: dict | None = None, split_waits: bool = True):
    """Build the per-core Bass program (same program on all 8 cores).

    reps > 1 replicates the whole computation back-to-back inside one NEFF
    (output overwritten each rep) — used only for differential timing."""
    opts = dict(_DEFAULT_OPTS, **(opts or {}))
    import concourse.bass as bass
    import concourse.tile as tile
    from concourse import mybir

    f32 = mybir.dt.float32
    f16 = mybir.dt.float16

    nk1a = HALF if sparse else NKE   # mm1 K-tiles for odd0 m-tiles
    nk2b = HALF if sparse else NKO   # mm2 K-tiles for even1 m-tiles

    nc = bass.Bass("TRN2", target_bir_lowering=False, debug=False)

    io_dt = f16 if opts["io_f16"] else f32
    sT = nc.dram_tensor("sT", [NKE, P, BC], io_dt, kind="ExternalInput")
    uT = nc.dram_tensor("uT", [HALF, P, BC], io_dt, kind="ExternalInput")
    w1a = nc.dram_tensor("w1a", [HALF, P, nk1a, P], f16, kind="ExternalInput")
    w1b = nc.dram_tensor("w1b", [HALF, P, NKE, P], f16, kind="ExternalInput")
    w2a = nc.dram_tensor("w2a", [HALF, P, NKO, P], f16, kind="ExternalInput")
    w2b = nc.dram_tensor("w2b", [HALF, P, nk2b, P], f16, kind="ExternalInput")
    bias1 = nc.dram_tensor("bias1", [P, NM1], f32, kind="ExternalInput")
    bias2 = nc.dram_tensor("bias2", [P, NM2], f32, kind="ExternalInput")
    out = nc.dram_tensor("o", [NM2, P, BC], f32, kind="ExternalOutput")

    with tile.TileContext(nc) as tc:
        with (
            tc.tile_pool(name="consts", bufs=1) as consts,
            tc.tile_pool(name="stage", bufs=opts["stage_bufs"]) as stage,
            tc.tile_pool(name="at", bufs=NKE) as at_pool,
            tc.tile_pool(name="ut", bufs=HALF) as ut_pool,
            tc.tile_pool(name="a2", bufs=NKO) as a2_pool,
            tc.tile_pool(name="wpool", bufs=opts["wpool_bufs"]) as wpool,
            tc.tile_pool(name="psum", bufs=opts["psum_bufs"], space="PSUM") as psum_pool,
            tc.tile_pool(name="opool", bufs=4) as opool,
        ):
            b1 = consts.tile([P, NM1], f32, tag="b1")
            nc.sync.dma_start(out=b1, in_=bias1[:, :])
            b2 = consts.tile([P, NM2], f32, tag="b2")
            nc.sync.dma_start(out=b2, in_=bias2[:, :])
            bneg2 = consts.tile([P, 1], f32, tag="bneg2")
            nc.vector.memset(bneg2, -2.0)

            pools = dict(
                stage=stage, at_pool=at_pool, ut_pool=ut_pool,
                a2_pool=a2_pool, wpool=wpool, psum_pool=psum_pool,
                opool=opool,
            )
            drams = dict(
                sT=sT, uT=uT, w1a=w1a, w1b=w1b, w2a=w2a, w2b=w2b, out=out
            )
            biases = dict(b1=b1, b2=b2, bneg2=bneg2)
            for _rep in range(reps):
                _kernel_body(nc, mybir, sparse, pools, drams, biases, opts)
    if opts["fuse_ldw"]:
        _patch_ldw_opt()
        _fuse_ldweights(nc)
    if split_waits:
        _split_excess_waits(nc, 1)
    return nc


def _kernel_body(nc, mybir, sparse, pools, drams, biases, opts):
    f32 = mybir.dt.float32
    f16 = mybir.dt.float16
    nk1a = HALF if sparse else NKE
    nk2b = HALF if sparse else NKO
    stage, at_pool, ut_pool, a2_pool, wpool, psum_pool, opool = (
        pools["stage"], pools["at_pool"], pools["ut_pool"], pools["a2_pool"],
        pools["wpool"], pools["psum_pool"], pools["opool"],
    )
    sT, uT, w1a, w1b, w2a, w2b, out = (
        drams["sT"], drams["uT"], drams["w1a"], drams["w1b"], drams["w2a"],
        drams["w2b"], drams["out"],
    )
    b1, b2, bneg2 = biases["b1"], biases["b2"], biases["bneg2"]
    io_dt = f16 if opts["io_f16"] else f32
    act_dma = nc.scalar if opts["ring_split"] else nc.sync
    strip_dma = nc.gpsimd if opts["strip_gpsimd"] else nc.sync

    # DMA ring split: weight strips go on the SP HWDGE ring (nc.sync),
    # activations/outputs on the ACT HWDGE ring (nc.scalar) — so the bulk
    # sT/uT loads never head-of-line-block the strip prefetch FIFO.

    # rho(s).T tiles, fp16, resident: AT[k] = sigmoid(4*sT[k] - 2)
    AT = []
    for k in range(NKE):
        st = stage.tile([P, BC], io_dt, tag="stage")
        act_dma.dma_start(out=st, in_=sT[k])
        a = at_pool.tile([P, BC], f16, tag="at")
        nc.scalar.activation(
            a, st, mybir.ActivationFunctionType.Sigmoid,
            bias=bneg2[:, 0:1], scale=4.0,
        )
        AT.append(a)

    # Ux.T tiles, fp32, resident
    UT = []
    for k in range(HALF):
        u = ut_pool.tile([P, BC], io_dt, tag="ut")
        act_dma.dma_start(out=u, in_=uT[k])
        UT.append(u)

    # ---- mm1: S1[odd,:] = W.T @ AT ; A2 = rho(S1 + b_odd [+ U]) ----
    # odd0 first: those m-tiles contract only over even0 (AT[0:16]), so the
    # PE can start after ~1/3 of the sT load instead of all of it.
    A2 = [None] * NM1
    mm1_order = (list(range(NM1)) if opts["mm1_odd0_first"]
                 else list(range(HALF, NM1)) + list(range(HALF)))
    for m in mm1_order:
        if m >= HALF:
            wt = wpool.tile([P, NKE, P], f16, tag="w")
            strip_dma.dma_start(out=wt, in_=w1b[m - HALF])
            ks = range(NKE)
        else:
            wt = wpool.tile([P, nk1a, P], f16, tag="w")
            strip_dma.dma_start(out=wt, in_=w1a[m])
            ks = range(nk1a)
        ps = psum_pool.tile([P, BC], f32, tag="ps")
        nkl = len(ks)
        for i, k in enumerate(ks):
            nc.tensor.matmul(
                ps, lhsT=wt[:, i, :], rhs=AT[k],
                start=(i == 0), stop=(i == nkl - 1),
            )
        if m < HALF:
            nc.vector.tensor_add(ps, ps, UT[m])
        a2 = a2_pool.tile([P, BC], f16, tag="a2")
        nc.scalar.activation(
            a2, ps, mybir.ActivationFunctionType.Sigmoid,
            bias=b1[:, m : m + 1], scale=4.0,
        )
        A2[m] = a2

    # ---- mm2: O[even,:] = W @ A2 + b_even ----
    # even1 first (small strips, deps = A2[16:] = the tail of mm1).
    for m in list(range(HALF, NM2)) + list(range(HALF)):
        if m >= HALF:
            wt = wpool.tile([P, nk2b, P], f16, tag="w")
            strip_dma.dma_start(out=wt, in_=w2b[m - HALF])
            ks = range(NKO - nk2b, NKO)
        else:
            wt = wpool.tile([P, NKO, P], f16, tag="w")
            strip_dma.dma_start(out=wt, in_=w2a[m])
            ks = range(NKO)
        ps = psum_pool.tile([P, BC], f32, tag="ps")
        nkl = len(ks)
        for i, k in enumerate(ks):
            nc.tensor.matmul(
                ps, lhsT=wt[:, i, :], rhs=A2[k],
                start=(i == 0), stop=(i == nkl - 1),
            )
        ot = opool.tile([P, BC], f32, tag="ot")
        nc.scalar.activation(
            ot, ps, mybir.ActivationFunctionType.Identity,
            bias=b2[:, m : m + 1], scale=1.0,
        )
        act_dma.dma_start(out=out[m], in_=ot)


def _strips(Wsub: np.ndarray, nm: int) -> np.ndarray:
    """[K, nm*128] -> [nm, 128, K//128, 128] contiguous per-m-tile K-strips.

    strip[j, p, kt, c] = Wsub[kt*128 + p, j*128 + c], so strip[j][:, kt, :]
    is the [K=128, M=128] lhsT tile for output tile j, contraction tile kt.
    """
    K = Wsub.shape[0]
    return np.ascontiguousarray(
        Wsub.reshape(K // P, P, nm, P).transpose(2, 1, 0, 3)
    )


def prepare_in_maps(inputs: dict, W: np.ndarray, sparse: bool, io_f16: bool = True) -> list:
    """Host-side prep: mask+cast+tile weights, transpose activations, shard."""
    f32 = np.float32
    s = np.asarray(inputs["s"], f32)
    Ux = np.asarray(inputs["Ux"], f32)
    assert s.shape == (B, E) and Ux.shape == (B, D1), (s.shape, Ux.shape)

    W16 = W.astype(np.float16)
    WT16 = np.ascontiguousarray(W16.T)

    if sparse:
        w1a = _strips(W16[:D1, :D1], HALF)
        w2b = _strips(WT16[D1:, D1:], HALF)
    else:
        w1a = _strips(W16[:, :D1], HALF)
        w2b = _strips(WT16[:, D1:], HALF)
    w1b = _strips(W16[:, D1:], HALF)
    w2a = _strips(WT16[:, :D1], HALF)

    bias1 = np.ascontiguousarray(
        (4.0 * np.asarray(inputs["b_odd"], f32).reshape(-1) - 2.0).reshape(NM1, P).T
    )
    bias2 = np.ascontiguousarray(
        np.asarray(inputs["b_even"], f32).reshape(-1).reshape(NM2, P).T
    )

    io_dt = np.float16 if io_f16 else f32
    sT_full = np.ascontiguousarray(s.T.astype(io_dt))   # [E, B]
    uT_full = np.ascontiguousarray(Ux.T.astype(io_dt))  # [D1, B]

    in_maps = []
    for c in range(NC):
        sl = slice(c * BC, (c + 1) * BC)
        in_maps.append({
            "sT": np.ascontiguousarray(sT_full[:, sl]).reshape(NKE, P, BC),
            "uT": np.ascontiguousarray(uT_full[:, sl]).reshape(HALF, P, BC),
            "w1a": w1a, "w1b": w1b, "w2a": w2a, "w2b": w2b,
            "bias1": bias1, "bias2": bias2,
        })
    return in_maps


def kernel(Ux, s, W_tensor, b_even, b_odd, W_mask):
    from concourse.bass_utils import run_bass_kernel_spmd

    f32 = np.float32
    W = np.asarray(W_tensor, f32) * np.asarray(W_mask, f32)
    sparse = not W[D1:, :D1].any()

    in_maps = prepare_in_maps(
        {"s": s, "Ux": Ux, "b_odd": b_odd, "b_even": b_even}, W, sparse,
        io_f16=_DEFAULT_OPTS["io_f16"],
    )

    nc = _KERNEL_CACHE.get(sparse)
    if nc is None:
        nc = _build(sparse)
        _KERNEL_CACHE[sparse] = nc

    res = run_bass_kernel_spmd(nc, in_maps, core_ids=list(range(NC)))
    out_T = np.concatenate(
        [res.results[c]["o"].reshape(E, BC) for c in range(NC)], axis=1
    )  # [E, B]
    return np.ascontiguousarray(out_T.T)



# revision 8
# speedup vs baseline: 49277.0276x; 48652.4141x over previous
"""Trainium2 Bass kernel for nn_EvenOddFunctionHAM.

Computes, for W = W_tensor * W_mask (block-staircase 4096x4096):
    s_odd = rho(s) @ W + b_odd;  s_odd[:, :2048] += Ux
    out   = rho(s_odd) @ W.T + b_even
with rho(x) = sigmoid(4x - 2).

Strategy: data-parallel over the batch (4096 rows -> 8 cores x 512).
Everything runs in a transposed layout (feature dim on SBUF partitions,
batch on the free axis) so no on-device transposes are needed:
    S1 = W.T @ rho(s).T   (contraction over the even dim)
    O  = W  @ rho(S1+..)  (contraction over the odd dim)
Weights are masked, cast to fp16, transposed, and pre-tiled into
contiguous per-m-tile K-strips on the host; matmuls run in fp16 with
fp32 PSUM accumulation. The staircase zero block is skipped when the
masked W actually has it (checked on host), saving 25% of the FLOPs.
"""

import numpy as np

_KERNEL_CACHE = {}

_DEFAULT_OPTS = {
    "ring_split": True,
    "mm1_odd0_first": True,
    "psum_bufs": 8,
    # fuse_ldw=True re-fuses Ldweights into self-loading matmuls and enables
    # walrus --enable-ldw-opt. Measured perf-neutral on this kernel (the PE
    # stream is not LDW-bound), so keep the default, battle-tested compile
    # path.
    "fuse_ldw": False,
    "wpool_bufs": 3,
    "stage_bufs": 4,
    "strip_gpsimd": False,
    # Stream s.T / Ux.T as fp16 (saves 6 MiB/core HBM traffic). Measured
    # perf-neutral at R=25 (554 vs 561 us) with slightly worse rel err
    # (3.27e-4 vs 3.08e-4), so keep fp32 inputs.
    "io_f16": False,
    # v2 schedule: fp16 in/out streams, resident stage pool, uT before the
    # even1 half of sT, and a2 activations interleaved ahead of the even1 AT
    # conversions so PSUM banks free promptly (the v1 act-queue FIFO stalled
    # the PE ~16us on the first rep waiting on stage DMAs). Steady-state
    # slope is identical to v1 (both at the fp16 PE roofline); v2 wins on
    # single-invocation latency and streams 10MB/core less HBM per call.
    "v2": True,
    "out_f16": True,
}

# ---- model dims (hardcoded per contract; asserted against inputs) ----
B = 4096        # batch
E = 4096        # even dim (rows of W)
O_DIM = 4096    # odd dim (cols of W)
D1 = 2048       # width of Ux / first odd block
NC = 8          # cores
BC = B // NC    # batch per core = 512
P = 128         # partitions
NKE = E // P    # 32 k-tiles over even
NKO = O_DIM // P
NM1 = O_DIM // P  # mm1 output tiles (odd)
NM2 = E // P      # mm2 output tiles (even)
HALF = D1 // P    # 16


def _split_excess_waits(nc, maxw: int = 1) -> int:
    """This walrus build encodes at most one sem wait per instruction, but
    Tile's scheduler can attach several. Move the overflow onto inserted
    same-engine NoOps directly preceding the instruction (engines are
    in-order, so consecutive waits are equivalent to one multi-wait)."""
    from concourse import mybir

    n = 0
    for f in nc.m.functions:
        for bb in f.blocks:
            insts = bb.instructions
            new = []
            for inst in insts:
                si = getattr(inst, "sync_info", None)
                if si is not None and len(si.on_wait) > maxw:
                    waits = list(si.on_wait)
                    over, keep = waits[:-maxw], waits[-maxw:]
                    for j in range(0, len(over), maxw):
                        n += 1
                        new.append(mybir.InstNoOp(
                            name=f"{inst.name}-ws{j}",
                            engine=inst.engine,
                            bass_nofuse=True,
                            sync_info=mybir.SyncInfo(
                                on_wait=over[j : j + maxw], on_update=[]
                            ),
                        ))
                    inst.sync_info = mybir.SyncInfo(
                        on_wait=keep, on_update=list(si.on_update)
                    )
                new.append(inst)
            if len(new) != len(insts):
                insts[:] = new
                assert len(bb.instructions) == len(new)
    return n



_LDW_PATCHED = False


def _patch_ldw_opt():
    """Compile with walrus --enable-ldw-opt=true (the concourse default
    pins it false). Requires self-loading matmuls (no explicit
    InstLdweights), which _fuse_ldweights produces."""
    global _LDW_PATCHED
    if _LDW_PATCHED:
        return
    from concourse import bass_utils
    _orig = bass_utils.run_command

    def _patched(argv, **kwargs):
        argv = ["--enable-ldw-opt=true" if a == "--enable-ldw-opt=false" else a
                for a in argv]
        return _orig(argv, **kwargs)

    bass_utils.run_command = _patched
    _LDW_PATCHED = True


def _fuse_ldweights(nc) -> int:
    """Tile legalization splits each matmul into InstLdweights + InstMatmult.
    Walrus's LDW optimization (fast weight load + pipelining) only applies to
    self-loading matmuls, so re-fuse: drop the Ldweights, move its sem waits
    onto the matmul, set ldweights=True."""
    from concourse import mybir

    n = 0
    for f in nc.m.functions:
        for bb in f.blocks:
            insts = bb.instructions
            new, pending = [], None
            for inst in insts:
                tn = type(inst).__name__
                if tn == "InstLdweights":
                    assert pending is None
                    pending = inst
                    continue
                if tn == "InstMatmult" and pending is not None:
                    si_l, si_m = pending.sync_info, inst.sync_info
                    waits = list(si_l.on_wait if si_l else []) + \
                        list(si_m.on_wait if si_m else [])
                    ups = list(si_l.on_update if si_l else []) + \
                        list(si_m.on_update if si_m else [])
                    inst.sync_info = mybir.SyncInfo(on_wait=waits, on_update=ups)
                    inst.ldweights = True
                    pending = None
                    n += 1
                new.append(inst)
            assert pending is None
            if len(new) != len(insts):
                insts[:] = new
    return n


def _build(sparse: bool, reps: int = 1, opts: dict | None = None, split_waits: bool = True):
    """Build the per-core Bass program (same program on all 8 cores).

    reps > 1 replicates the whole computation back-to-back inside one NEFF
    (output overwritten each rep) — used only for differential timing."""
    opts = dict(_DEFAULT_OPTS, **(opts or {}))
    import concourse.bass as bass
    import concourse.tile as tile
    from concourse import mybir

    f32 = mybir.dt.float32
    f16 = mybir.dt.float16

    nk1a = HALF if sparse else NKE   # mm1 K-tiles for odd0 m-tiles
    nk2b = HALF if sparse else NKO   # mm2 K-tiles for even1 m-tiles

    nc = bass.Bass("TRN2", target_bir_lowering=False, debug=False)

    io_dt = f16 if (opts["io_f16"] or opts["v2"]) else f32
    out_dt = f16 if (opts["v2"] and opts["out_f16"]) else f32
    sT = nc.dram_tensor("sT", [NKE, P, BC], io_dt, kind="ExternalInput")
    uT = nc.dram_tensor("uT", [HALF, P, BC], io_dt, kind="ExternalInput")
    w1a = nc.dram_tensor("w1a", [HALF, P, nk1a, P], f16, kind="ExternalInput")
    w1b = nc.dram_tensor("w1b", [HALF, P, NKE, P], f16, kind="ExternalInput")
    w2a = nc.dram_tensor("w2a", [HALF, P, NKO, P], f16, kind="ExternalInput")
    w2b = nc.dram_tensor("w2b", [HALF, P, nk2b, P], f16, kind="ExternalInput")
    bias1 = nc.dram_tensor("bias1", [P, NM1], f32, kind="ExternalInput")
    bias2 = nc.dram_tensor("bias2", [P, NM2], f32, kind="ExternalInput")
    out = nc.dram_tensor("o", [NM2, P, BC], out_dt, kind="ExternalOutput")

    stage_bufs = NKE if opts["v2"] else opts["stage_bufs"]
    with tile.TileContext(nc) as tc:
        with (
            tc.tile_pool(name="consts", bufs=1) as consts,
            tc.tile_pool(name="stage", bufs=stage_bufs) as stage,
            tc.tile_pool(name="at", bufs=NKE) as at_pool,
            tc.tile_pool(name="ut", bufs=HALF) as ut_pool,
            tc.tile_pool(name="a2", bufs=NKO) as a2_pool,
            tc.tile_pool(name="wpool", bufs=opts["wpool_bufs"]) as wpool,
            tc.tile_pool(name="psum", bufs=opts["psum_bufs"], space="PSUM") as psum_pool,
            tc.tile_pool(name="opool", bufs=4) as opool,
        ):
            b1 = consts.tile([P, NM1], f32, tag="b1")
            nc.sync.dma_start(out=b1, in_=bias1[:, :])
            b2 = consts.tile([P, NM2], f32, tag="b2")
            nc.sync.dma_start(out=b2, in_=bias2[:, :])
            bneg2 = consts.tile([P, 1], f32, tag="bneg2")
            nc.vector.memset(bneg2, -2.0)

            pools = dict(
                stage=stage, at_pool=at_pool, ut_pool=ut_pool,
                a2_pool=a2_pool, wpool=wpool, psum_pool=psum_pool,
                opool=opool,
            )
            drams = dict(
                sT=sT, uT=uT, w1a=w1a, w1b=w1b, w2a=w2a, w2b=w2b, out=out
            )
            biases = dict(b1=b1, b2=b2, bneg2=bneg2)
            body = _kernel_body_v2 if opts["v2"] else _kernel_body
            for _rep in range(reps):
                body(nc, mybir, sparse, pools, drams, biases, opts)
    if opts["fuse_ldw"]:
        _patch_ldw_opt()
        _fuse_ldweights(nc)
    if split_waits:
        _split_excess_waits(nc, 1)
    return nc


def _kernel_body(nc, mybir, sparse, pools, drams, biases, opts):
    f32 = mybir.dt.float32
    f16 = mybir.dt.float16
    nk1a = HALF if sparse else NKE
    nk2b = HALF if sparse else NKO
    stage, at_pool, ut_pool, a2_pool, wpool, psum_pool, opool = (
        pools["stage"], pools["at_pool"], pools["ut_pool"], pools["a2_pool"],
        pools["wpool"], pools["psum_pool"], pools["opool"],
    )
    sT, uT, w1a, w1b, w2a, w2b, out = (
        drams["sT"], drams["uT"], drams["w1a"], drams["w1b"], drams["w2a"],
        drams["w2b"], drams["out"],
    )
    b1, b2, bneg2 = biases["b1"], biases["b2"], biases["bneg2"]
    io_dt = f16 if opts["io_f16"] else f32
    act_dma = nc.scalar if opts["ring_split"] else nc.sync
    strip_dma = nc.gpsimd if opts["strip_gpsimd"] else nc.sync

    # DMA ring split: weight strips go on the SP HWDGE ring (nc.sync),
    # activations/outputs on the ACT HWDGE ring (nc.scalar) — so the bulk
    # sT/uT loads never head-of-line-block the strip prefetch FIFO.

    # rho(s).T tiles, fp16, resident: AT[k] = sigmoid(4*sT[k] - 2)
    AT = []
    for k in range(NKE):
        st = stage.tile([P, BC], io_dt, tag="stage")
        act_dma.dma_start(out=st, in_=sT[k])
        a = at_pool.tile([P, BC], f16, tag="at")
        nc.scalar.activation(
            a, st, mybir.ActivationFunctionType.Sigmoid,
            bias=bneg2[:, 0:1], scale=4.0,
        )
        AT.append(a)

    # Ux.T tiles, fp32, resident
    UT = []
    for k in range(HALF):
        u = ut_pool.tile([P, BC], io_dt, tag="ut")
        act_dma.dma_start(out=u, in_=uT[k])
        UT.append(u)

    # ---- mm1: S1[odd,:] = W.T @ AT ; A2 = rho(S1 + b_odd [+ U]) ----
    # odd0 first: those m-tiles contract only over even0 (AT[0:16]), so the
    # PE can start after ~1/3 of the sT load instead of all of it.
    A2 = [None] * NM1
    mm1_order = (list(range(NM1)) if opts["mm1_odd0_first"]
                 else list(range(HALF, NM1)) + list(range(HALF)))
    for m in mm1_order:
        if m >= HALF:
            wt = wpool.tile([P, NKE, P], f16, tag="w")
            strip_dma.dma_start(out=wt, in_=w1b[m - HALF])
            ks = range(NKE)
        else:
            wt = wpool.tile([P, nk1a, P], f16, tag="w")
            strip_dma.dma_start(out=wt, in_=w1a[m])
            ks = range(nk1a)
        ps = psum_pool.tile([P, BC], f32, tag="ps")
        nkl = len(ks)
        for i, k in enumerate(ks):
            nc.tensor.matmul(
                ps, lhsT=wt[:, i, :], rhs=AT[k],
                start=(i == 0), stop=(i == nkl - 1),
            )
        if m < HALF:
            nc.vector.tensor_add(ps, ps, UT[m])
        a2 = a2_pool.tile([P, BC], f16, tag="a2")
        nc.scalar.activation(
            a2, ps, mybir.ActivationFunctionType.Sigmoid,
            bias=b1[:, m : m + 1], scale=4.0,
        )
        A2[m] = a2

    # ---- mm2: O[even,:] = W @ A2 + b_even ----
    # even1 first (small strips, deps = A2[16:] = the tail of mm1).
    for m in list(range(HALF, NM2)) + list(range(HALF)):
        if m >= HALF:
            wt = wpool.tile([P, nk2b, P], f16, tag="w")
            strip_dma.dma_start(out=wt, in_=w2b[m - HALF])
            ks = range(NKO - nk2b, NKO)
        else:
            wt = wpool.tile([P, NKO, P], f16, tag="w")
            strip_dma.dma_start(out=wt, in_=w2a[m])
            ks = range(NKO)
        ps = psum_pool.tile([P, BC], f32, tag="ps")
        nkl = len(ks)
        for i, k in enumerate(ks):
            nc.tensor.matmul(
                ps, lhsT=wt[:, i, :], rhs=A2[k],
                start=(i == 0), stop=(i == nkl - 1),
            )
        ot = opool.tile([P, BC], f32, tag="ot")
        nc.scalar.activation(
            ot, ps, mybir.ActivationFunctionType.Identity,
            bias=b2[:, m : m + 1], scale=1.0,
        )
        act_dma.dma_start(out=out[m], in_=ot)


def _kernel_body_v2(nc, mybir, sparse, pools, drams, biases, opts):
    """Restructured schedule: the v1 ACT-queue FIFO carried stage DMAs that
    wait on buffer reuse ahead of the a2 activations that free PSUM banks,
    stalling the PE ~16us/rep. Here the stage pool is fully resident (DMA
    issues never wait), uT loads ride between the two sT halves (they gate
    the Ux add at each odd0 chain end), and each odd0 chain's a2 activation
    is emitted before the next even1 AT conversion."""
    f32 = mybir.dt.float32
    f16 = mybir.dt.float16
    nk1a = HALF if sparse else NKE
    nk2b = HALF if sparse else NKO
    stage, at_pool, ut_pool, a2_pool, wpool, psum_pool, opool = (
        pools["stage"], pools["at_pool"], pools["ut_pool"], pools["a2_pool"],
        pools["wpool"], pools["psum_pool"], pools["opool"],
    )
    sT, uT, w1a, w1b, w2a, w2b, out = (
        drams["sT"], drams["uT"], drams["w1a"], drams["w1b"], drams["w2a"],
        drams["w2b"], drams["out"],
    )
    b1, b2, bneg2 = biases["b1"], biases["b2"], biases["bneg2"]
    out_dt = f16 if opts["out_f16"] else f32
    sig = mybir.ActivationFunctionType.Sigmoid

    # -- input DMA issues (ACT HWDGE ring), consumption order --
    ST = [stage.tile([P, BC], f16, tag="stage", name=f"st{k}")
          for k in range(NKE)]
    UT = [ut_pool.tile([P, BC], f16, tag="ut", name=f"ut{m}")
          for m in range(HALF)]
    for k in range(HALF):
        nc.scalar.dma_start(out=ST[k], in_=sT[k])
    for m in range(HALF):
        nc.scalar.dma_start(out=UT[m], in_=uT[m])
    for k in range(HALF, NKE):
        nc.scalar.dma_start(out=ST[k], in_=sT[k])

    # AT conversions for the even0 half
    AT = [None] * NKE
    for k in range(HALF):
        a = at_pool.tile([P, BC], f16, tag="at")
        nc.scalar.activation(a, ST[k], sig, bias=bneg2[:, 0:1], scale=4.0)
        AT[k] = a

    # ---- mm1 odd0 chains; one even1 AT conversion after each a2 ----
    A2 = [None] * NM1
    for m in range(HALF):
        wt = wpool.tile([P, nk1a, P], f16, tag="w")
        nc.sync.dma_start(out=wt, in_=w1a[m])
        ps = psum_pool.tile([P, BC], f32, tag="ps")
        for i in range(nk1a):
            nc.tensor.matmul(
                ps, lhsT=wt[:, i, :], rhs=AT[i],
                start=(i == 0), stop=(i == nk1a - 1),
            )
        nc.vector.tensor_add(ps, ps, UT[m])
        a2 = a2_pool.tile([P, BC], f16, tag="a2")
        nc.scalar.activation(a2, ps, sig, bias=b1[:, m : m + 1], scale=4.0)
        A2[m] = a2
        k2 = HALF + m
        a = at_pool.tile([P, BC], f16, tag="at")
        nc.scalar.activation(a, ST[k2], sig, bias=bneg2[:, 0:1], scale=4.0)
        AT[k2] = a

    # ---- mm1 odd1 chains (contract all 32 k-tiles) ----
    for m in range(HALF, NM1):
        wt = wpool.tile([P, NKE, P], f16, tag="w")
        nc.sync.dma_start(out=wt, in_=w1b[m - HALF])
        ps = psum_pool.tile([P, BC], f32, tag="ps")
        for i in range(NKE):
            nc.tensor.matmul(
                ps, lhsT=wt[:, i, :], rhs=AT[i],
                start=(i == 0), stop=(i == NKE - 1),
            )
        a2 = a2_pool.tile([P, BC], f16, tag="a2")
        nc.scalar.activation(a2, ps, sig, bias=b1[:, m : m + 1], scale=4.0)
        A2[m] = a2

    # ---- mm2: even1 first (short strips, deps = tail of mm1) ----
    for m in list(range(HALF, NM2)) + list(range(HALF)):
        if m >= HALF:
            wt = wpool.tile([P, nk2b, P], f16, tag="w")
            nc.sync.dma_start(out=wt, in_=w2b[m - HALF])
            ks = range(NKO - nk2b, NKO)
        else:
            wt = wpool.tile([P, NKO, P], f16, tag="w")
            nc.sync.dma_start(out=wt, in_=w2a[m])
            ks = range(NKO)
        ps = psum_pool.tile([P, BC], f32, tag="ps")
        nkl = len(ks)
        for i, k in enumerate(ks):
            nc.tensor.matmul(
                ps, lhsT=wt[:, i, :], rhs=A2[k],
                start=(i == 0), stop=(i == nkl - 1),
            )
        ot = opool.tile([P, BC], out_dt, tag="ot")
        nc.scalar.activation(
            ot, ps, mybir.ActivationFunctionType.Identity,
            bias=b2[:, m : m + 1], scale=1.0,
        )
        nc.scalar.dma_start(out=out[m], in_=ot)


def _strips(Wsub: np.ndarray, nm: int) -> np.ndarray:
    """[K, nm*128] -> [nm, 128, K//128, 128] contiguous per-m-tile K-strips.

    strip[j, p, kt, c] = Wsub[kt*128 + p, j*128 + c], so strip[j][:, kt, :]
    is the [K=128, M=128] lhsT tile for output tile j, contraction tile kt.
    """
    K = Wsub.shape[0]
    return np.ascontiguousarray(
        Wsub.reshape(K // P, P, nm, P).transpose(2, 1, 0, 3)
    )


def prepare_in_maps(inputs: dict, W: np.ndarray, sparse: bool, io_f16: bool = True) -> list:
    """Host-side prep: mask+cast+tile weights, transpose activations, shard."""
    f32 = np.float32
    s = np.asarray(inputs["s"], f32)
    Ux = np.asarray(inputs["Ux"], f32)
    assert s.shape == (B, E) and Ux.shape == (B, D1), (s.shape, Ux.shape)

    W16 = W.astype(np.float16)
    WT16 = np.ascontiguousarray(W16.T)

    if sparse:
        w1a = _strips(W16[:D1, :D1], HALF)
        w2b = _strips(WT16[D1:, D1:], HALF)
    else:
        w1a = _strips(W16[:, :D1], HALF)
        w2b = _strips(WT16[:, D1:], HALF)
    w1b = _strips(W16[:, D1:], HALF)
    w2a = _strips(WT16[:, :D1], HALF)

    bias1 = np.ascontiguousarray(
        (4.0 * np.asarray(inputs["b_odd"], f32).reshape(-1) - 2.0).reshape(NM1, P).T
    )
    bias2 = np.ascontiguousarray(
        np.asarray(inputs["b_even"], f32).reshape(-1).reshape(NM2, P).T
    )

    io_dt = np.float16 if io_f16 else f32
    sT_full = np.ascontiguousarray(s.T.astype(io_dt))   # [E, B]
    uT_full = np.ascontiguousarray(Ux.T.astype(io_dt))  # [D1, B]

    in_maps = []
    for c in range(NC):
        sl = slice(c * BC, (c + 1) * BC)
        in_maps.append({
            "sT": np.ascontiguousarray(sT_full[:, sl]).reshape(NKE, P, BC),
            "uT": np.ascontiguousarray(uT_full[:, sl]).reshape(HALF, P, BC),
            "w1a": w1a, "w1b": w1b, "w2a": w2a, "w2b": w2b,
            "bias1": bias1, "bias2": bias2,
        })
    return in_maps


def kernel(Ux, s, W_tensor, b_even, b_odd, W_mask):
    from concourse.bass_utils import run_bass_kernel_spmd

    f32 = np.float32
    W = np.asarray(W_tensor, f32) * np.asarray(W_mask, f32)
    sparse = not W[D1:, :D1].any()

    opts = dict(_DEFAULT_OPTS)
    in_maps = prepare_in_maps(
        {"s": s, "Ux": Ux, "b_odd": b_odd, "b_even": b_even}, W, sparse,
        io_f16=(opts["io_f16"] or opts["v2"]),
    )

    nc = _KERNEL_CACHE.get(sparse)
    if nc is None:
        nc = _build(sparse)
        _KERNEL_CACHE[sparse] = nc

    res = run_bass_kernel_spmd(nc, in_maps, core_ids=list(range(NC)))
    out_T = np.concatenate(
        [res.results[c]["o"].reshape(E, BC).astype(f32) for c in range(NC)],
        axis=1,
    )  # [E, B]
    return np.ascontiguousarray(out_T.T)



# revision 9
# speedup vs baseline: 51280.3940x; 1.0407x over previous
"""Trainium2 Bass kernel for nn_EvenOddFunctionHAM.

Computes, for W = W_tensor * W_mask (block-staircase 4096x4096):
    s_odd = rho(s) @ W + b_odd;  s_odd[:, :2048] += Ux
    out   = rho(s_odd) @ W.T + b_even
with rho(x) = sigmoid(4x - 2).

Strategy: data-parallel over the batch (4096 rows -> 8 cores x 512).
Everything runs in a transposed layout (feature dim on SBUF partitions,
batch on the free axis) so no on-device transposes are needed:
    S1 = W.T @ rho(s).T   (contraction over the even dim)
    O  = W  @ rho(S1+..)  (contraction over the odd dim)
Weights are masked, cast to fp16, transposed, and pre-tiled into
contiguous per-m-tile K-strips on the host; matmuls run in fp16 with
fp32 PSUM accumulation. The staircase zero block is skipped when the
masked W actually has it (checked on host), saving 25% of the FLOPs.

Steady state runs at the fp16 PE roofline: 1536 matmuls x 512 cols x
(1/2.4GHz) = 328us/core, measured ~330-350us (burst) / ~405us when the
PE streak exceeds the sustained-power envelope. fp8 DoubleRow cannot
beat this: the 2e-2 gate needs a hi/lo split (3 chains/layer), and DR's
256-col LDWEIGHTS stream at 1.2GHz alone equals the fp16 MM floor.

The v2 schedule (default) streams s.T/Ux.T in fp16 and the output in
fp16 (cast back to f32 on host), keeps the input stage pool fully
resident so stage DMA issues never wait, loads uT between the two sT
halves (uT gates the Ux add at each odd0 chain end), and interleaves
each odd0 chain's a2 activation ahead of the next even1 AT conversion
so PSUM banks free promptly. This removes the ~16us first-rep PE stall
the v1 ACT-queue FIFO caused; per-rep steady state is unchanged.
"""

import numpy as np

_KERNEL_CACHE = {}

_DEFAULT_OPTS = {
    "ring_split": True,
    "mm1_odd0_first": True,
    "psum_bufs": 8,
    # fuse_ldw=True re-fuses Ldweights into self-loading matmuls and enables
    # walrus --enable-ldw-opt. Measured perf-neutral on this kernel (the PE
    # stream is not LDW-bound), so keep the default, battle-tested compile
    # path.
    "fuse_ldw": False,
    "wpool_bufs": 3,
    "stage_bufs": 4,
    "strip_gpsimd": False,
    # Stream s.T / Ux.T as fp16 (saves 6 MiB/core HBM traffic). Measured
    # perf-neutral at R=25 (554 vs 561 us) with slightly worse rel err
    # (3.27e-4 vs 3.08e-4), so keep fp32 inputs.
    "io_f16": False,
    # v2 schedule: fp16 in/out streams, resident stage pool, uT before the
    # even1 half of sT, and a2 activations interleaved ahead of the even1 AT
    # conversions so PSUM banks free promptly (the v1 act-queue FIFO stalled
    # the PE ~16us on the first rep waiting on stage DMAs). Steady-state
    # slope is identical to v1 (both at the fp16 PE roofline); v2 wins on
    # single-invocation latency and streams 10MB/core less HBM per call.
    "v2": True,
    "out_f16": True,
}

# ---- model dims (hardcoded per contract; asserted against inputs) ----
B = 4096        # batch
E = 4096        # even dim (rows of W)
O_DIM = 4096    # odd dim (cols of W)
D1 = 2048       # width of Ux / first odd block
NC = 8          # cores
BC = B // NC    # batch per core = 512
P = 128         # partitions
NKE = E // P    # 32 k-tiles over even
NKO = O_DIM // P
NM1 = O_DIM // P  # mm1 output tiles (odd)
NM2 = E // P      # mm2 output tiles (even)
HALF = D1 // P    # 16


def _split_excess_waits(nc, maxw: int = 1) -> int:
    """This walrus build encodes at most one sem wait per instruction, but
    Tile's scheduler can attach several. Move the overflow onto inserted
    same-engine NoOps directly preceding the instruction (engines are
    in-order, so consecutive waits are equivalent to one multi-wait)."""
    from concourse import mybir

    n = 0
    for f in nc.m.functions:
        for bb in f.blocks:
            insts = bb.instructions
            new = []
            for inst in insts:
                si = getattr(inst, "sync_info", None)
                if si is not None and len(si.on_wait) > maxw:
                    waits = list(si.on_wait)
                    over, keep = waits[:-maxw], waits[-maxw:]
                    for j in range(0, len(over), maxw):
                        n += 1
                        new.append(mybir.InstNoOp(
                            name=f"{inst.name}-ws{j}",
                            engine=inst.engine,
                            bass_nofuse=True,
                            sync_info=mybir.SyncInfo(
                                on_wait=over[j : j + maxw], on_update=[]
                            ),
                        ))
                    inst.sync_info = mybir.SyncInfo(
                        on_wait=keep, on_update=list(si.on_update)
                    )
                new.append(inst)
            if len(new) != len(insts):
                insts[:] = new
                assert len(bb.instructions) == len(new)
    return n



_LDW_PATCHED = False


def _patch_ldw_opt():
    """Compile with walrus --enable-ldw-opt=true (the concourse default
    pins it false). Requires self-loading matmuls (no explicit
    InstLdweights), which _fuse_ldweights produces."""
    global _LDW_PATCHED
    if _LDW_PATCHED:
        return
    from concourse import bass_utils
    _orig = bass_utils.run_command

    def _patched(argv, **kwargs):
        argv = ["--enable-ldw-opt=true" if a == "--enable-ldw-opt=false" else a
                for a in argv]
        return _orig(argv, **kwargs)

    bass_utils.run_command = _patched
    _LDW_PATCHED = True


def _fuse_ldweights(nc) -> int:
    """Tile legalization splits each matmul into InstLdweights + InstMatmult.
    Walrus's LDW optimization (fast weight load + pipelining) only applies to
    self-loading matmuls, so re-fuse: drop the Ldweights, move its sem waits
    onto the matmul, set ldweights=True."""
    from concourse import mybir

    n = 0
    for f in nc.m.functions:
        for bb in f.blocks:
            insts = bb.instructions
            new, pending = [], None
            for inst in insts:
                tn = type(inst).__name__
                if tn == "InstLdweights":
                    assert pending is None
                    pending = inst
                    continue
                if tn == "InstMatmult" and pending is not None:
                    si_l, si_m = pending.sync_info, inst.sync_info
                    waits = list(si_l.on_wait if si_l else []) + \
                        list(si_m.on_wait if si_m else [])
                    ups = list(si_l.on_update if si_l else []) + \
                        list(si_m.on_update if si_m else [])
                    inst.sync_info = mybir.SyncInfo(on_wait=waits, on_update=ups)
                    inst.ldweights = True
                    pending = None
                    n += 1
                new.append(inst)
            assert pending is None
            if len(new) != len(insts):
                insts[:] = new
    return n


def _build(sparse: bool, reps: int = 1, opts: dict | None = None, split_waits: bool = True):
    """Build the per-core Bass program (same program on all 8 cores).

    reps > 1 replicates the whole computation back-to-back inside one NEFF
    (output overwritten each rep) — used only for differential timing."""
    opts = dict(_DEFAULT_OPTS, **(opts or {}))
    import concourse.bass as bass
    import concourse.tile as tile
    from concourse import mybir

    f32 = mybir.dt.float32
    f16 = mybir.dt.float16

    nk1a = HALF if sparse else NKE   # mm1 K-tiles for odd0 m-tiles
    nk2b = HALF if sparse else NKO   # mm2 K-tiles for even1 m-tiles

    nc = bass.Bass("TRN2", target_bir_lowering=False, debug=False)

    io_dt = f16 if (opts["io_f16"] or opts["v2"]) else f32
    out_dt = f16 if (opts["v2"] and opts["out_f16"]) else f32
    sT = nc.dram_tensor("sT", [NKE, P, BC], io_dt, kind="ExternalInput")
    uT = nc.dram_tensor("uT", [HALF, P, BC], io_dt, kind="ExternalInput")
    w1a = nc.dram_tensor("w1a", [HALF, P, nk1a, P], f16, kind="ExternalInput")
    w1b = nc.dram_tensor("w1b", [HALF, P, NKE, P], f16, kind="ExternalInput")
    w2a = nc.dram_tensor("w2a", [HALF, P, NKO, P], f16, kind="ExternalInput")
    w2b = nc.dram_tensor("w2b", [HALF, P, nk2b, P], f16, kind="ExternalInput")
    bias1 = nc.dram_tensor("bias1", [P, NM1], f32, kind="ExternalInput")
    bias2 = nc.dram_tensor("bias2", [P, NM2], f32, kind="ExternalInput")
    out = nc.dram_tensor("o", [NM2, P, BC], out_dt, kind="ExternalOutput")

    stage_bufs = NKE if opts["v2"] else opts["stage_bufs"]
    with tile.TileContext(nc) as tc:
        with (
            tc.tile_pool(name="consts", bufs=1) as consts,
            tc.tile_pool(name="stage", bufs=stage_bufs) as stage,
            tc.tile_pool(name="at", bufs=NKE) as at_pool,
            tc.tile_pool(name="ut", bufs=HALF) as ut_pool,
            tc.tile_pool(name="a2", bufs=NKO) as a2_pool,
            tc.tile_pool(name="wpool", bufs=opts["wpool_bufs"]) as wpool,
            tc.tile_pool(name="psum", bufs=opts["psum_bufs"], space="PSUM") as psum_pool,
            tc.tile_pool(name="opool", bufs=4) as opool,
        ):
            b1 = consts.tile([P, NM1], f32, tag="b1")
            nc.sync.dma_start(out=b1, in_=bias1[:, :])
            b2 = consts.tile([P, NM2], f32, tag="b2")
            nc.sync.dma_start(out=b2, in_=bias2[:, :])
            bneg2 = consts.tile([P, 1], f32, tag="bneg2")
            nc.vector.memset(bneg2, -2.0)

            pools = dict(
                stage=stage, at_pool=at_pool, ut_pool=ut_pool,
                a2_pool=a2_pool, wpool=wpool, psum_pool=psum_pool,
                opool=opool,
            )
            drams = dict(
                sT=sT, uT=uT, w1a=w1a, w1b=w1b, w2a=w2a, w2b=w2b, out=out
            )
            biases = dict(b1=b1, b2=b2, bneg2=bneg2)
            body = _kernel_body_v2 if opts["v2"] else _kernel_body
            for _rep in range(reps):
                body(nc, mybir, sparse, pools, drams, biases, opts)
    if opts["fuse_ldw"]:
        _patch_ldw_opt()
        _fuse_ldweights(nc)
    if split_waits:
        _split_excess_waits(nc, 1)
    return nc


def _kernel_body(nc, mybir, sparse, pools, drams, biases, opts):
    f32 = mybir.dt.float32
    f16 = mybir.dt.float16
    nk1a = HALF if sparse else NKE
    nk2b = HALF if sparse else NKO
    stage, at_pool, ut_pool, a2_pool, wpool, psum_pool, opool = (
        pools["stage"], pools["at_pool"], pools["ut_pool"], pools["a2_pool"],
        pools["wpool"], pools["psum_pool"], pools["opool"],
    )
    sT, uT, w1a, w1b, w2a, w2b, out = (
        drams["sT"], drams["uT"], drams["w1a"], drams["w1b"], drams["w2a"],
        drams["w2b"], drams["out"],
    )
    b1, b2, bneg2 = biases["b1"], biases["b2"], biases["bneg2"]
    io_dt = f16 if opts["io_f16"] else f32
    act_dma = nc.scalar if opts["ring_split"] else nc.sync
    strip_dma = nc.gpsimd if opts["strip_gpsimd"] else nc.sync

    # DMA ring split: weight strips go on the SP HWDGE ring (nc.sync),
    # activations/outputs on the ACT HWDGE ring (nc.scalar) — so the bulk
    # sT/uT loads never head-of-line-block the strip prefetch FIFO.

    # rho(s).T tiles, fp16, resident: AT[k] = sigmoid(4*sT[k] - 2)
    AT = []
    for k in range(NKE):
        st = stage.tile([P, BC], io_dt, tag="stage")
        act_dma.dma_start(out=st, in_=sT[k])
        a = at_pool.tile([P, BC], f16, tag="at")
        nc.scalar.activation(
            a, st, mybir.ActivationFunctionType.Sigmoid,
            bias=bneg2[:, 0:1], scale=4.0,
        )
        AT.append(a)

    # Ux.T tiles, fp32, resident
    UT = []
    for k in range(HALF):
        u = ut_pool.tile([P, BC], io_dt, tag="ut")
        act_dma.dma_start(out=u, in_=uT[k])
        UT.append(u)

    # ---- mm1: S1[odd,:] = W.T @ AT ; A2 = rho(S1 + b_odd [+ U]) ----
    # odd0 first: those m-tiles contract only over even0 (AT[0:16]), so the
    # PE can start after ~1/3 of the sT load instead of all of it.
    A2 = [None] * NM1
    mm1_order = (list(range(NM1)) if opts["mm1_odd0_first"]
                 else list(range(HALF, NM1)) + list(range(HALF)))
    for m in mm1_order:
        if m >= HALF:
            wt = wpool.tile([P, NKE, P], f16, tag="w")
            strip_dma.dma_start(out=wt, in_=w1b[m - HALF])
            ks = range(NKE)
        else:
            wt = wpool.tile([P, nk1a, P], f16, tag="w")
            strip_dma.dma_start(out=wt, in_=w1a[m])
            ks = range(nk1a)
        ps = psum_pool.tile([P, BC], f32, tag="ps")
        nkl = len(ks)
        for i, k in enumerate(ks):
            nc.tensor.matmul(
                ps, lhsT=wt[:, i, :], rhs=AT[k],
                start=(i == 0), stop=(i == nkl - 1),
            )
        if m < HALF:
            nc.vector.tensor_add(ps, ps, UT[m])
        a2 = a2_pool.tile([P, BC], f16, tag="a2")
        nc.scalar.activation(
            a2, ps, mybir.ActivationFunctionType.Sigmoid,
            bias=b1[:, m : m + 1], scale=4.0,
        )
        A2[m] = a2

    # ---- mm2: O[even,:] = W @ A2 + b_even ----
    # even1 first (small strips, deps = A2[16:] = the tail of mm1).
    for m in list(range(HALF, NM2)) + list(range(HALF)):
        if m >= HALF:
            wt = wpool.tile([P, nk2b, P], f16, tag="w")
            strip_dma.dma_start(out=wt, in_=w2b[m - HALF])
            ks = range(NKO - nk2b, NKO)
        else:
            wt = wpool.tile([P, NKO, P], f16, tag="w")
            strip_dma.dma_start(out=wt, in_=w2a[m])
            ks = range(NKO)
        ps = psum_pool.tile([P, BC], f32, tag="ps")
        nkl = len(ks)
        for i, k in enumerate(ks):
            nc.tensor.matmul(
                ps, lhsT=wt[:, i, :], rhs=A2[k],
                start=(i == 0), stop=(i == nkl - 1),
            )
        ot = opool.tile([P, BC], f32, tag="ot")
        nc.scalar.activation(
            ot, ps, mybir.ActivationFunctionType.Identity,
            bias=b2[:, m : m + 1], scale=1.0,
        )
        act_dma.dma_start(out=out[m], in_=ot)


def _kernel_body_v2(nc, mybir, sparse, pools, drams, biases, opts):
    """Restructured schedule: the v1 ACT-queue FIFO carried stage DMAs that
    wait on buffer reuse ahead of the a2 activations that free PSUM banks,
    stalling the PE ~16us/rep. Here the stage pool is fully resident (DMA
    issues never wait), uT loads ride between the two sT halves (they gate
    the Ux add at each odd0 chain end), and each odd0 chain's a2 activation
    is emitted before the next even1 AT conversion."""
    f32 = mybir.dt.float32
    f16 = mybir.dt.float16
    nk1a = HALF if sparse else NKE
    nk2b = HALF if sparse else NKO
    stage, at_pool, ut_pool, a2_pool, wpool, psum_pool, opool = (
        pools["stage"], pools["at_pool"], pools["ut_pool"], pools["a2_pool"],
        pools["wpool"], pools["psum_pool"], pools["opool"],
    )
    sT, uT, w1a, w1b, w2a, w2b, out = (
        drams["sT"], drams["uT"], drams["w1a"], drams["w1b"], drams["w2a"],
        drams["w2b"], drams["out"],
    )
    b1, b2, bneg2 = biases["b1"], biases["b2"], biases["bneg2"]
    out_dt = f16 if opts["out_f16"] else f32
    sig = mybir.ActivationFunctionType.Sigmoid

    # -- input DMA issues (ACT HWDGE ring), consumption order --
    ST = [stage.tile([P, BC], f16, tag="stage", name=f"st{k}")
          for k in range(NKE)]
    UT = [ut_pool.tile([P, BC], f16, tag="ut", name=f"ut{m}")
          for m in range(HALF)]
    for k in range(HALF):
        nc.scalar.dma_start(out=ST[k], in_=sT[k])
    for m in range(HALF):
        nc.scalar.dma_start(out=UT[m], in_=uT[m])
    for k in range(HALF, NKE):
        nc.scalar.dma_start(out=ST[k], in_=sT[k])

    # AT conversions for the even0 half
    AT = [None] * NKE
    for k in range(HALF):
        a = at_pool.tile([P, BC], f16, tag="at")
        nc.scalar.activation(a, ST[k], sig, bias=bneg2[:, 0:1], scale=4.0)
        AT[k] = a

    # ---- mm1 odd0 chains; one even1 AT conversion after each a2 ----
    A2 = [None] * NM1
    for m in range(HALF):
        wt = wpool.tile([P, nk1a, P], f16, tag="w")
        nc.sync.dma_start(out=wt, in_=w1a[m])
        ps = psum_pool.tile([P, BC], f32, tag="ps")
        for i in range(nk1a):
            nc.tensor.matmul(
                ps, lhsT=wt[:, i, :], rhs=AT[i],
                start=(i == 0), stop=(i == nk1a - 1),
            )
        nc.vector.tensor_add(ps, ps, UT[m])
        a2 = a2_pool.tile([P, BC], f16, tag="a2")
        nc.scalar.activation(a2, ps, sig, bias=b1[:, m : m + 1], scale=4.0)
        A2[m] = a2
        k2 = HALF + m
        a = at_pool.tile([P, BC], f16, tag="at")
        nc.scalar.activation(a, ST[k2], sig, bias=bneg2[:, 0:1], scale=4.0)
        AT[k2] = a

    # ---- mm1 odd1 chains (contract all 32 k-tiles) ----
    for m in range(HALF, NM1):
        wt = wpool.tile([P, NKE, P], f16, tag="w")
        nc.sync.dma_start(out=wt, in_=w1b[m - HALF])
        ps = psum_pool.tile([P, BC], f32, tag="ps")
        for i in range(NKE):
            nc.tensor.matmul(
                ps, lhsT=wt[:, i, :], rhs=AT[i],
                start=(i == 0), stop=(i == NKE - 1),
            )
        a2 = a2_pool.tile([P, BC], f16, tag="a2")
        nc.scalar.activation(a2, ps, sig, bias=b1[:, m : m + 1], scale=4.0)
        A2[m] = a2

    # ---- mm2: even1 first (short strips, deps = tail of mm1) ----
    for m in list(range(HALF, NM2)) + list(range(HALF)):
        if m >= HALF:
            wt = wpool.tile([P, nk2b, P], f16, tag="w")
            nc.sync.dma_start(out=wt, in_=w2b[m - HALF])
            ks = range(NKO - nk2b, NKO)
        else:
            wt = wpool.tile([P, NKO, P], f16, tag="w")
            nc.sync.dma_start(out=wt, in_=w2a[m])
            ks = range(NKO)
        ps = psum_pool.tile([P, BC], f32, tag="ps")
        nkl = len(ks)
        for i, k in enumerate(ks):
            nc.tensor.matmul(
                ps, lhsT=wt[:, i, :], rhs=A2[k],
                start=(i == 0), stop=(i == nkl - 1),
            )
        ot = opool.tile([P, BC], out_dt, tag="ot")
        nc.scalar.activation(
            ot, ps, mybir.ActivationFunctionType.Identity,
            bias=b2[:, m : m + 1], scale=1.0,
        )
        nc.scalar.dma_start(out=out[m], in_=ot)


def _strips(Wsub: np.ndarray, nm: int) -> np.ndarray:
    """[K, nm*128] -> [nm, 128, K//128, 128] contiguous per-m-tile K-strips.

    strip[j, p, kt, c] = Wsub[kt*128 + p, j*128 + c], so strip[j][:, kt, :]
    is the [K=128, M=128] lhsT tile for output tile j, contraction tile kt.
    """
    K = Wsub.shape[0]
    return np.ascontiguousarray(
        Wsub.reshape(K // P, P, nm, P).transpose(2, 1, 0, 3)
    )


def prepare_in_maps(inputs: dict, W: np.ndarray, sparse: bool, io_f16: bool = True) -> list:
    """Host-side prep: mask+cast+tile weights, transpose activations, shard."""
    f32 = np.float32
    s = np.asarray(inputs["s"], f32)
    Ux = np.asarray(inputs["Ux"], f32)
    assert s.shape == (B, E) and Ux.shape == (B, D1), (s.shape, Ux.shape)

    W16 = W.astype(np.float16)
    WT16 = np.ascontiguousarray(W16.T)

    if sparse:
        w1a = _strips(W16[:D1, :D1], HALF)
        w2b = _strips(WT16[D1:, D1:], HALF)
    else:
        w1a = _strips(W16[:, :D1], HALF)
        w2b = _strips(WT16[:, D1:], HALF)
    w1b = _strips(W16[:, D1:], HALF)
    w2a = _strips(WT16[:, :D1], HALF)

    bias1 = np.ascontiguousarray(
        (4.0 * np.asarray(inputs["b_odd"], f32).reshape(-1) - 2.0).reshape(NM1, P).T
    )
    bias2 = np.ascontiguousarray(
        np.asarray(inputs["b_even"], f32).reshape(-1).reshape(NM2, P).T
    )

    io_dt = np.float16 if io_f16 else f32
    sT_full = np.ascontiguousarray(s.T.astype(io_dt))   # [E, B]
    uT_full = np.ascontiguousarray(Ux.T.astype(io_dt))  # [D1, B]

    in_maps = []
    for c in range(NC):
        sl = slice(c * BC, (c + 1) * BC)
        in_maps.append({
            "sT": np.ascontiguousarray(sT_full[:, sl]).reshape(NKE, P, BC),
            "uT": np.ascontiguousarray(uT_full[:, sl]).reshape(HALF, P, BC),
            "w1a": w1a, "w1b": w1b, "w2a": w2a, "w2b": w2b,
            "bias1": bias1, "bias2": bias2,
        })
    return in_maps


def kernel(Ux, s, W_tensor, b_even, b_odd, W_mask):
    from concourse.bass_utils import run_bass_kernel_spmd

    f32 = np.float32
    W = np.asarray(W_tensor, f32) * np.asarray(W_mask, f32)
    sparse = not W[D1:, :D1].any()

    opts = dict(_DEFAULT_OPTS)
    in_maps = prepare_in_maps(
        {"s": s, "Ux": Ux, "b_odd": b_odd, "b_even": b_even}, W, sparse,
        io_f16=(opts["io_f16"] or opts["v2"]),
    )

    nc = _KERNEL_CACHE.get(sparse)
    if nc is None:
        nc = _build(sparse)
        _KERNEL_CACHE[sparse] = nc

    res = run_bass_kernel_spmd(nc, in_maps, core_ids=list(range(NC)))
    out_T = np.concatenate(
        [res.results[c]["o"].reshape(E, BC).astype(f32) for c in range(NC)],
        axis=1,
    )  # [E, B]
    return np.ascontiguousarray(out_T.T)



# revision 11
# speedup vs baseline: 52802.4716x; 1.0297x over previous
"""Trainium2 Bass kernel for nn_EvenOddFunctionHAM.

Computes, for W = W_tensor * W_mask (block-staircase 4096x4096):
    s_odd = rho(s) @ W + b_odd;  s_odd[:, :2048] += Ux
    out   = rho(s_odd) @ W.T + b_even
with rho(x) = sigmoid(4x - 2).

Strategy: data-parallel over the batch (4096 rows -> 8 cores x 512).
Everything runs in a transposed layout (feature dim on SBUF partitions,
batch on the free axis) so no on-device transposes are needed:
    S1 = W.T @ rho(s).T   (contraction over the even dim)
    O  = W  @ rho(S1+..)  (contraction over the odd dim)
Weights are masked, cast to fp16, transposed, and pre-tiled into
contiguous per-m-tile K-strips on the host; matmuls run in fp16 with
fp32 PSUM accumulation. The staircase zero block is skipped when the
masked W actually has it (checked on host), saving 25% of the FLOPs.

Steady state runs at the fp16 PE roofline: 1536 matmuls x 512 cols x
(1/2.4GHz) = 328us/core, measured ~330-350us (burst) / ~405us when the
PE streak exceeds the sustained-power envelope. fp8 DoubleRow cannot
beat this: the 2e-2 gate needs a hi/lo split (3 chains/layer), and DR's
256-col LDWEIGHTS stream at 1.2GHz alone equals the fp16 MM floor.

The v2 schedule (default) streams s.T/Ux.T in fp16 and the output in
fp16 (cast back to f32 on host), keeps the input stage pool fully
resident so stage DMA issues never wait, loads uT between the two sT
halves (uT gates the Ux add at each odd0 chain end), and interleaves
each odd0 chain's a2 activation ahead of the next even1 AT conversion
so PSUM banks free promptly. This removes the ~16us first-rep PE stall
the v1 ACT-queue FIFO caused; per-rep steady state is unchanged.
"""

import numpy as np

_KERNEL_CACHE = {}

_DEFAULT_OPTS = {
    "ring_split": True,
    "mm1_odd0_first": True,
    "psum_bufs": 8,
    # fuse_ldw=True re-fuses Ldweights into self-loading matmuls and enables
    # walrus --enable-ldw-opt. Measured perf-neutral on this kernel (the PE
    # stream is not LDW-bound), so keep the default, battle-tested compile
    # path.
    "fuse_ldw": False,
    # fuse_only: re-fuse Ldweights into self-loading matmuls WITHOUT the
    # walrus flag monkeypatch (no framework mutation) — halves the PE-queue
    # instruction count.
    "fuse_only": False,
    "wpool_bufs": 3,
    "stage_bufs": 4,
    "strip_gpsimd": False,
    # Stream s.T / Ux.T as fp16 (saves 6 MiB/core HBM traffic). Measured
    # perf-neutral at R=25 (554 vs 561 us) with slightly worse rel err
    # (3.27e-4 vs 3.08e-4), so keep fp32 inputs.
    "io_f16": False,
    # v2 schedule: fp16 in/out streams, resident stage pool, uT before the
    # even1 half of sT, and a2 activations interleaved ahead of the even1 AT
    # conversions so PSUM banks free promptly (the v1 act-queue FIFO stalled
    # the PE ~16us on the first rep waiting on stage DMAs). Steady-state
    # slope is identical to v1 (both at the fp16 PE roofline); v2 wins on
    # single-invocation latency and streams 10MB/core less HBM per call.
    "v2": True,
    "out_f16": True,
}

# ---- model dims (hardcoded per contract; asserted against inputs) ----
B = 4096        # batch
E = 4096        # even dim (rows of W)
O_DIM = 4096    # odd dim (cols of W)
D1 = 2048       # width of Ux / first odd block
NC = 8          # cores
BC = B // NC    # batch per core = 512
P = 128         # partitions
NKE = E // P    # 32 k-tiles over even
NKO = O_DIM // P
NM1 = O_DIM // P  # mm1 output tiles (odd)
NM2 = E // P      # mm2 output tiles (even)
HALF = D1 // P    # 16


def _split_excess_waits(nc, maxw: int = 1) -> int:
    """This walrus build encodes at most one sem wait per instruction, but
    Tile's scheduler can attach several. Move the overflow onto inserted
    same-engine NoOps directly preceding the instruction (engines are
    in-order, so consecutive waits are equivalent to one multi-wait)."""
    from concourse import mybir

    n = 0
    for f in nc.m.functions:
        for bb in f.blocks:
            insts = bb.instructions
            new = []
            for inst in insts:
                si = getattr(inst, "sync_info", None)
                if si is not None and len(si.on_wait) > maxw:
                    waits = list(si.on_wait)
                    over, keep = waits[:-maxw], waits[-maxw:]
                    for j in range(0, len(over), maxw):
                        n += 1
                        new.append(mybir.InstNoOp(
                            name=f"{inst.name}-ws{j}",
                            engine=inst.engine,
                            bass_nofuse=True,
                            sync_info=mybir.SyncInfo(
                                on_wait=over[j : j + maxw], on_update=[]
                            ),
                        ))
                    inst.sync_info = mybir.SyncInfo(
                        on_wait=keep, on_update=list(si.on_update)
                    )
                new.append(inst)
            if len(new) != len(insts):
                insts[:] = new
                assert len(bb.instructions) == len(new)
    return n



_LDW_PATCHED = False


def _patch_ldw_opt():
    """Compile with walrus --enable-ldw-opt=true (the concourse default
    pins it false). Requires self-loading matmuls (no explicit
    InstLdweights), which _fuse_ldweights produces."""
    global _LDW_PATCHED
    if _LDW_PATCHED:
        return
    from concourse import bass_utils
    _orig = bass_utils.run_command

    def _patched(argv, **kwargs):
        argv = ["--enable-ldw-opt=true" if a == "--enable-ldw-opt=false" else a
                for a in argv]
        return _orig(argv, **kwargs)

    bass_utils.run_command = _patched
    _LDW_PATCHED = True


def _fuse_ldweights(nc) -> int:
    """Tile legalization splits each matmul into InstLdweights + InstMatmult.
    Walrus's LDW optimization (fast weight load + pipelining) only applies to
    self-loading matmuls, so re-fuse: drop the Ldweights, move its sem waits
    onto the matmul, set ldweights=True."""
    from concourse import mybir

    n = 0
    for f in nc.m.functions:
        for bb in f.blocks:
            insts = bb.instructions
            new, pending = [], None
            for inst in insts:
                tn = type(inst).__name__
                if tn == "InstLdweights":
                    assert pending is None
                    pending = inst
                    continue
                if tn == "InstMatmult" and pending is not None:
                    si_l, si_m = pending.sync_info, inst.sync_info
                    waits = list(si_l.on_wait if si_l else []) + \
                        list(si_m.on_wait if si_m else [])
                    ups = list(si_l.on_update if si_l else []) + \
                        list(si_m.on_update if si_m else [])
                    inst.sync_info = mybir.SyncInfo(on_wait=waits, on_update=ups)
                    inst.ldweights = True
                    pending = None
                    n += 1
                new.append(inst)
            assert pending is None
            if len(new) != len(insts):
                insts[:] = new
    return n


def _build(sparse: bool, reps: int = 1, opts: dict | None = None, split_waits: bool = True):
    """Build the per-core Bass program (same program on all 8 cores).

    reps > 1 replicates the whole computation back-to-back inside one NEFF
    (output overwritten each rep) — used only for differential timing."""
    opts = dict(_DEFAULT_OPTS, **(opts or {}))
    import concourse.bass as bass
    import concourse.tile as tile
    from concourse import mybir

    f32 = mybir.dt.float32
    f16 = mybir.dt.float16

    nk1a = HALF if sparse else NKE   # mm1 K-tiles for odd0 m-tiles
    nk2b = HALF if sparse else NKO   # mm2 K-tiles for even1 m-tiles

    nc = bass.Bass("TRN2", target_bir_lowering=False, debug=False)

    io_dt = f16 if (opts["io_f16"] or opts["v2"]) else f32
    out_dt = f16 if (opts["v2"] and opts["out_f16"]) else f32
    sT = nc.dram_tensor("sT", [NKE, P, BC], io_dt, kind="ExternalInput")
    uT = nc.dram_tensor("uT", [HALF, P, BC], io_dt, kind="ExternalInput")
    w1a = nc.dram_tensor("w1a", [HALF, P, nk1a, P], f16, kind="ExternalInput")
    w1b = nc.dram_tensor("w1b", [HALF, P, NKE, P], f16, kind="ExternalInput")
    w2a = nc.dram_tensor("w2a", [HALF, P, NKO, P], f16, kind="ExternalInput")
    w2b = nc.dram_tensor("w2b", [HALF, P, nk2b, P], f16, kind="ExternalInput")
    bias1 = nc.dram_tensor("bias1", [P, NM1], f32, kind="ExternalInput")
    bias2 = nc.dram_tensor("bias2", [P, NM2], f32, kind="ExternalInput")
    out = nc.dram_tensor("o", [NM2, P, BC], out_dt, kind="ExternalOutput")

    stage_bufs = NKE if opts["v2"] else opts["stage_bufs"]
    with tile.TileContext(nc) as tc:
        with (
            tc.tile_pool(name="consts", bufs=1) as consts,
            tc.tile_pool(name="stage", bufs=stage_bufs) as stage,
            tc.tile_pool(name="at", bufs=NKE) as at_pool,
            tc.tile_pool(name="ut", bufs=HALF) as ut_pool,
            tc.tile_pool(name="a2", bufs=NKO) as a2_pool,
            tc.tile_pool(name="wpool", bufs=opts["wpool_bufs"]) as wpool,
            tc.tile_pool(name="psum", bufs=opts["psum_bufs"], space="PSUM") as psum_pool,
            tc.tile_pool(name="opool", bufs=4) as opool,
        ):
            b1 = consts.tile([P, NM1], f32, tag="b1")
            nc.sync.dma_start(out=b1, in_=bias1[:, :])
            b2 = consts.tile([P, NM2], f32, tag="b2")
            nc.sync.dma_start(out=b2, in_=bias2[:, :])
            bneg2 = consts.tile([P, 1], f32, tag="bneg2")
            nc.vector.memset(bneg2, -2.0)

            pools = dict(
                stage=stage, at_pool=at_pool, ut_pool=ut_pool,
                a2_pool=a2_pool, wpool=wpool, psum_pool=psum_pool,
                opool=opool,
            )
            drams = dict(
                sT=sT, uT=uT, w1a=w1a, w1b=w1b, w2a=w2a, w2b=w2b, out=out
            )
            biases = dict(b1=b1, b2=b2, bneg2=bneg2)
            body = _kernel_body_v2 if opts["v2"] else _kernel_body
            for _rep in range(reps):
                body(nc, mybir, sparse, pools, drams, biases, opts)
    if opts["fuse_ldw"]:
        _patch_ldw_opt()
        _fuse_ldweights(nc)
    elif opts["fuse_only"]:
        _fuse_ldweights(nc)
    if split_waits:
        _split_excess_waits(nc, 1)
    return nc


def _kernel_body(nc, mybir, sparse, pools, drams, biases, opts):
    f32 = mybir.dt.float32
    f16 = mybir.dt.float16
    nk1a = HALF if sparse else NKE
    nk2b = HALF if sparse else NKO
    stage, at_pool, ut_pool, a2_pool, wpool, psum_pool, opool = (
        pools["stage"], pools["at_pool"], pools["ut_pool"], pools["a2_pool"],
        pools["wpool"], pools["psum_pool"], pools["opool"],
    )
    sT, uT, w1a, w1b, w2a, w2b, out = (
        drams["sT"], drams["uT"], drams["w1a"], drams["w1b"], drams["w2a"],
        drams["w2b"], drams["out"],
    )
    b1, b2, bneg2 = biases["b1"], biases["b2"], biases["bneg2"]
    io_dt = f16 if opts["io_f16"] else f32
    act_dma = nc.scalar if opts["ring_split"] else nc.sync
    strip_dma = nc.gpsimd if opts["strip_gpsimd"] else nc.sync

    # DMA ring split: weight strips go on the SP HWDGE ring (nc.sync),
    # activations/outputs on the ACT HWDGE ring (nc.scalar) — so the bulk
    # sT/uT loads never head-of-line-block the strip prefetch FIFO.

    # rho(s).T tiles, fp16, resident: AT[k] = sigmoid(4*sT[k] - 2)
    AT = []
    for k in range(NKE):
        st = stage.tile([P, BC], io_dt, tag="stage")
        act_dma.dma_start(out=st, in_=sT[k])
        a = at_pool.tile([P, BC], f16, tag="at")
        nc.scalar.activation(
            a, st, mybir.ActivationFunctionType.Sigmoid,
            bias=bneg2[:, 0:1], scale=4.0,
        )
        AT.append(a)

    # Ux.T tiles, fp32, resident
    UT = []
    for k in range(HALF):
        u = ut_pool.tile([P, BC], io_dt, tag="ut")
        act_dma.dma_start(out=u, in_=uT[k])
        UT.append(u)

    # ---- mm1: S1[odd,:] = W.T @ AT ; A2 = rho(S1 + b_odd [+ U]) ----
    # odd0 first: those m-tiles contract only over even0 (AT[0:16]), so the
    # PE can start after ~1/3 of the sT load instead of all of it.
    A2 = [None] * NM1
    mm1_order = (list(range(NM1)) if opts["mm1_odd0_first"]
                 else list(range(HALF, NM1)) + list(range(HALF)))
    for m in mm1_order:
        if m >= HALF:
            wt = wpool.tile([P, NKE, P], f16, tag="w")
            strip_dma.dma_start(out=wt, in_=w1b[m - HALF])
            ks = range(NKE)
        else:
            wt = wpool.tile([P, nk1a, P], f16, tag="w")
            strip_dma.dma_start(out=wt, in_=w1a[m])
            ks = range(nk1a)
        ps = psum_pool.tile([P, BC], f32, tag="ps")
        nkl = len(ks)
        for i, k in enumerate(ks):
            nc.tensor.matmul(
                ps, lhsT=wt[:, i, :], rhs=AT[k],
                start=(i == 0), stop=(i == nkl - 1),
            )
        if m < HALF:
            nc.vector.tensor_add(ps, ps, UT[m])
        a2 = a2_pool.tile([P, BC], f16, tag="a2")
        nc.scalar.activation(
            a2, ps, mybir.ActivationFunctionType.Sigmoid,
            bias=b1[:, m : m + 1], scale=4.0,
        )
        A2[m] = a2

    # ---- mm2: O[even,:] = W @ A2 + b_even ----
    # even1 first (small strips, deps = A2[16:] = the tail of mm1).
    for m in list(range(HALF, NM2)) + list(range(HALF)):
        if m >= HALF:
            wt = wpool.tile([P, nk2b, P], f16, tag="w")
            strip_dma.dma_start(out=wt, in_=w2b[m - HALF])
            ks = range(NKO - nk2b, NKO)
        else:
            wt = wpool.tile([P, NKO, P], f16, tag="w")
            strip_dma.dma_start(out=wt, in_=w2a[m])
            ks = range(NKO)
        ps = psum_pool.tile([P, BC], f32, tag="ps")
        nkl = len(ks)
        for i, k in enumerate(ks):
            nc.tensor.matmul(
                ps, lhsT=wt[:, i, :], rhs=A2[k],
                start=(i == 0), stop=(i == nkl - 1),
            )
        ot = opool.tile([P, BC], f32, tag="ot")
        nc.scalar.activation(
            ot, ps, mybir.ActivationFunctionType.Identity,
            bias=b2[:, m : m + 1], scale=1.0,
        )
        act_dma.dma_start(out=out[m], in_=ot)


def _kernel_body_v2(nc, mybir, sparse, pools, drams, biases, opts):
    """Restructured schedule: the v1 ACT-queue FIFO carried stage DMAs that
    wait on buffer reuse ahead of the a2 activations that free PSUM banks,
    stalling the PE ~16us/rep. Here the stage pool is fully resident (DMA
    issues never wait), uT loads ride between the two sT halves (they gate
    the Ux add at each odd0 chain end), and each odd0 chain's a2 activation
    is emitted before the next even1 AT conversion."""
    f32 = mybir.dt.float32
    f16 = mybir.dt.float16
    nk1a = HALF if sparse else NKE
    nk2b = HALF if sparse else NKO
    stage, at_pool, ut_pool, a2_pool, wpool, psum_pool, opool = (
        pools["stage"], pools["at_pool"], pools["ut_pool"], pools["a2_pool"],
        pools["wpool"], pools["psum_pool"], pools["opool"],
    )
    sT, uT, w1a, w1b, w2a, w2b, out = (
        drams["sT"], drams["uT"], drams["w1a"], drams["w1b"], drams["w2a"],
        drams["w2b"], drams["out"],
    )
    b1, b2, bneg2 = biases["b1"], biases["b2"], biases["bneg2"]
    out_dt = f16 if opts["out_f16"] else f32
    sig = mybir.ActivationFunctionType.Sigmoid

    # -- input DMA issues (ACT HWDGE ring), consumption order --
    ST = [stage.tile([P, BC], f16, tag="stage", name=f"st{k}")
          for k in range(NKE)]
    UT = [ut_pool.tile([P, BC], f16, tag="ut", name=f"ut{m}")
          for m in range(HALF)]
    for k in range(HALF):
        nc.scalar.dma_start(out=ST[k], in_=sT[k])
    for m in range(HALF):
        nc.scalar.dma_start(out=UT[m], in_=uT[m])
    for k in range(HALF, NKE):
        nc.scalar.dma_start(out=ST[k], in_=sT[k])

    # AT conversions for the even0 half
    AT = [None] * NKE
    for k in range(HALF):
        a = at_pool.tile([P, BC], f16, tag="at")
        nc.scalar.activation(a, ST[k], sig, bias=bneg2[:, 0:1], scale=4.0)
        AT[k] = a

    # ---- mm1 odd0 chains; one even1 AT conversion after each a2 ----
    A2 = [None] * NM1
    for m in range(HALF):
        wt = wpool.tile([P, nk1a, P], f16, tag="w")
        nc.sync.dma_start(out=wt, in_=w1a[m])
        ps = psum_pool.tile([P, BC], f32, tag="ps")
        for i in range(nk1a):
            nc.tensor.matmul(
                ps, lhsT=wt[:, i, :], rhs=AT[i],
                start=(i == 0), stop=(i == nk1a - 1),
            )
        nc.vector.tensor_add(ps, ps, UT[m])
        a2 = a2_pool.tile([P, BC], f16, tag="a2")
        nc.scalar.activation(a2, ps, sig, bias=b1[:, m : m + 1], scale=4.0)
        A2[m] = a2
        k2 = HALF + m
        a = at_pool.tile([P, BC], f16, tag="at")
        nc.scalar.activation(a, ST[k2], sig, bias=bneg2[:, 0:1], scale=4.0)
        AT[k2] = a

    # ---- mm1 odd1 chains (contract all 32 k-tiles) ----
    for m in range(HALF, NM1):
        wt = wpool.tile([P, NKE, P], f16, tag="w")
        nc.sync.dma_start(out=wt, in_=w1b[m - HALF])
        ps = psum_pool.tile([P, BC], f32, tag="ps")
        for i in range(NKE):
            nc.tensor.matmul(
                ps, lhsT=wt[:, i, :], rhs=AT[i],
                start=(i == 0), stop=(i == NKE - 1),
            )
        a2 = a2_pool.tile([P, BC], f16, tag="a2")
        nc.scalar.activation(a2, ps, sig, bias=b1[:, m : m + 1], scale=4.0)
        A2[m] = a2

    # ---- mm2: even1 first (short strips, deps = tail of mm1) ----
    for m in list(range(HALF, NM2)) + list(range(HALF)):
        if m >= HALF:
            wt = wpool.tile([P, nk2b, P], f16, tag="w")
            nc.sync.dma_start(out=wt, in_=w2b[m - HALF])
            ks = range(NKO - nk2b, NKO)
        else:
            wt = wpool.tile([P, NKO, P], f16, tag="w")
            nc.sync.dma_start(out=wt, in_=w2a[m])
            ks = range(NKO)
        ps = psum_pool.tile([P, BC], f32, tag="ps")
        nkl = len(ks)
        for i, k in enumerate(ks):
            nc.tensor.matmul(
                ps, lhsT=wt[:, i, :], rhs=A2[k],
                start=(i == 0), stop=(i == nkl - 1),
            )
        ot = opool.tile([P, BC], out_dt, tag="ot")
        nc.scalar.activation(
            ot, ps, mybir.ActivationFunctionType.Identity,
            bias=b2[:, m : m + 1], scale=1.0,
        )
        nc.scalar.dma_start(out=out[m], in_=ot)


def _strips(Wsub: np.ndarray, nm: int) -> np.ndarray:
    """[K, nm*128] -> [nm, 128, K//128, 128] contiguous per-m-tile K-strips.

    strip[j, p, kt, c] = Wsub[kt*128 + p, j*128 + c], so strip[j][:, kt, :]
    is the [K=128, M=128] lhsT tile for output tile j, contraction tile kt.
    """
    K = Wsub.shape[0]
    return np.ascontiguousarray(
        Wsub.reshape(K // P, P, nm, P).transpose(2, 1, 0, 3)
    )


def prepare_in_maps(inputs: dict, W: np.ndarray, sparse: bool, io_f16: bool = True) -> list:
    """Host-side prep: mask+cast+tile weights, transpose activations, shard."""
    f32 = np.float32
    s = np.asarray(inputs["s"], f32)
    Ux = np.asarray(inputs["Ux"], f32)
    assert s.shape == (B, E) and Ux.shape == (B, D1), (s.shape, Ux.shape)

    W16 = W.astype(np.float16)
    WT16 = np.ascontiguousarray(W16.T)

    if sparse:
        w1a = _strips(W16[:D1, :D1], HALF)
        w2b = _strips(WT16[D1:, D1:], HALF)
    else:
        w1a = _strips(W16[:, :D1], HALF)
        w2b = _strips(WT16[:, D1:], HALF)
    w1b = _strips(W16[:, D1:], HALF)
    w2a = _strips(WT16[:, :D1], HALF)

    bias1 = np.ascontiguousarray(
        (4.0 * np.asarray(inputs["b_odd"], f32).reshape(-1) - 2.0).reshape(NM1, P).T
    )
    bias2 = np.ascontiguousarray(
        np.asarray(inputs["b_even"], f32).reshape(-1).reshape(NM2, P).T
    )

    io_dt = np.float16 if io_f16 else f32
    sT_full = np.ascontiguousarray(s.T.astype(io_dt))   # [E, B]
    uT_full = np.ascontiguousarray(Ux.T.astype(io_dt))  # [D1, B]

    in_maps = []
    for c in range(NC):
        sl = slice(c * BC, (c + 1) * BC)
        in_maps.append({
            "sT": np.ascontiguousarray(sT_full[:, sl]).reshape(NKE, P, BC),
            "uT": np.ascontiguousarray(uT_full[:, sl]).reshape(HALF, P, BC),
            "w1a": w1a, "w1b": w1b, "w2a": w2a, "w2b": w2b,
            "bias1": bias1, "bias2": bias2,
        })
    return in_maps


def kernel(Ux, s, W_tensor, b_even, b_odd, W_mask):
    from concourse.bass_utils import run_bass_kernel_spmd

    f32 = np.float32
    W = np.asarray(W_tensor, f32) * np.asarray(W_mask, f32)
    sparse = not W[D1:, :D1].any()

    opts = dict(_DEFAULT_OPTS)
    in_maps = prepare_in_maps(
        {"s": s, "Ux": Ux, "b_odd": b_odd, "b_even": b_even}, W, sparse,
        io_f16=(opts["io_f16"] or opts["v2"]),
    )

    nc = _KERNEL_CACHE.get(sparse)
    if nc is None:
        nc = _build(sparse)
        _KERNEL_CACHE[sparse] = nc

    res = run_bass_kernel_spmd(nc, in_maps, core_ids=list(range(NC)))
    out_T = np.concatenate(
        [res.results[c]["o"].reshape(E, BC).astype(f32) for c in range(NC)],
        axis=1,
    )  # [E, B]
    return np.ascontiguousarray(out_T.T)

